# revision 3
# baseline (speedup 1.0000x reference)
"""CrossModalAttention TRN2 kernel (v2: fp8 DoubleRow attention).

Strategy (data-parallel over batch, one batch element per NeuronCore):
  dir a: q from rgb, k/v from pl;  dir b: q from pl, k/v from rgb.
  Per direction:
    Q  = scale*(Wq @ f_q + bq)        [128 e, N] bf16 (scale folded into W,b)
    K  = Wk @ f_k + bk                [128 e, N] bf16
    VT = (Wv @ f_k)^T                 [N k, 128 e] fp8e4m3 (v-bias folded
                                      into the BN shift host-side)
    per q-tile (512 wide), per group g of 2 k-chunks:
      S^T_g = K_g^T @ Q_tile          [128 k, 2, 512 q]  (PSUM f32)
      E_g   = exp(S^T_g) -> fp8       ScalarE for most groups; VectorE
                                      computes e4m3 bits directly via the
                                      round(x*8*log2e + 55.5) affine trick
                                      for DVE_GROUPS (engine balance)
      OT   += VT_g^T @ E_g            one fp8 DoubleRow matmul (256-row
                                      contraction, 2x col rate)
      dn   += ones^T @ E_g            one fp8 DoubleRow matmul (weight padded
                                      to 16 cols for the lw step%16 rule),
                                      delayed 3 groups to stay off the
                                      critical path
      OT_norm = OT * bcast(1/dn)      reciprocal_approx_fast on DVE; bcast
                                      via Kc=1 rank-1 matmul
  y = Wp_a @ OT_a + Wp_b @ OT_b ; out = relu(inv*y + shift)  (BN folded)

Schedule: dir-a K/V features DMA first; dir-a projections then dir-a
attention start immediately, with dir-b projections slipped between the
first dir-a segments so the exp engines start ~40us earlier than a
proj-everything-first order.
"""

import sys

sys.path.insert(0, "/opt/trn_rl_repo")

import numpy as np

B = 8
C = 256
E = 128
OUT = 256
H = W = 64
N = H * W
QW = 512
SCALE = float(E) ** -0.5

LOG2E = 1.4426950408889634
FE8_A = 8.0 * LOG2E          # e4m3 bits = round(s*FE8_A + FE8_B)
FE8_B = 7.0 * 8.0 - 0.5      # HW float->uint8 rounds to nearest; c=-0.5
# groups (of 16 per segment) whose exp runs on DVE instead of ScalarE
DVE_GROUPS = frozenset({3, 7, 11, 14})

_CACHE = {}


def _patch_tail_drain(tile_mod, mybir):
    # This walrus build encodes Drain as CTRL_NO_STRUCT with a single
    # sync-wait slot; split the TileContext tail drain's waits across
    # one drain instruction per semaphore.
    if getattr(tile_mod.TileContext, "_drain_patched", False):
        return
    from concourse.vector_clock import ScopedClock

    def _drain_and_barrier(self, tick_clock, wait_clock):
        nc = self.nc
        drain_inst = nc.sync.drain()
        wait_clock.add_sem_waits(
            drain_inst.ins, ScopedClock({None: tick_clock.global_clock})
        )
        si = drain_inst.ins.sync_info
        if si is not None and si.on_wait and len(si.on_wait) > 1:
            waits = list(si.on_wait)
            drain_inst.ins.sync_info = mybir.SyncInfo(
                on_wait=[waits[0]], on_update=list(si.on_update or [])
            )
            for w in waits[1:]:
                d2 = nc.sync.drain()
                d2.ins.sync_info = mybir.SyncInfo(on_wait=[w], on_update=[])
        nc.all_engine_barrier()
        popped = nc._tile_sem_poison_stack.pop()
        assert popped is self._sem_poison
        nc.clear_and_free_semaphores(list(self.sems.allocated().values()))
        nc.all_engine_barrier()

    tile_mod.TileContext._drain_and_barrier = _drain_and_barrier
    tile_mod.TileContext._drain_patched = True


def build_nc(n=N, debug=False):
    """Build the single-core Bass program. n = spatial size (4096 full)."""
    import concourse.bacc as bacc
    import concourse.tile as tile
    from concourse import mybir

    f32 = mybir.dt.float32
    f32r = mybir.dt.float32r
    bf16 = mybir.dt.bfloat16
    fp8 = mybir.dt.float8e4
    u8 = mybir.dt.uint8
    AFT = mybir.ActivationFunctionType
    ALU = mybir.AluOpType
    DR = mybir.MatmulPerfMode.DoubleRow

    gj = 2                  # k-chunks per PSUM S-tile / exp instruction
    nqt = n // QW
    nkc = n // 128
    ngrp = nkc // gj        # exp groups per segment
    DN_DELAY = 3            # groups the dn matmul trails its exp by

    nc = bacc.Bacc(trn_type="TRN2", target_bir_lowering=False, debug=False)

    def din(name, shape, dt_=f32):
        return nc.dram_tensor(name, shape, dt_, kind="ExternalInput").ap()

    u16 = mybir.dt.uint16
    f_a_d = din("f_a", [C, n], u16)   # rgb features bf16 bits (q-side of a)
    f_b_d = din("f_b", [C, n], u16)   # pl features bf16 bits
    wq_a_d = din("wq_a", [C, E], u16)  # scale * W_q_rgb^T (bf16 bits)
    wk_a_d = din("wk_a", [C, E], u16)  # W_k_pl^T
    wv_a_d = din("wv_a", [C, E], u16)  # W_v_pl^T
    wq_b_d = din("wq_b", [C, E], u16)  # scale * W_q_pl^T
    wk_b_d = din("wk_b", [C, E], u16)  # W_k_rgb^T
    wv_b_d = din("wv_b", [C, E], u16)  # W_v_rgb^T
    wp_d = din("wp", [2 * E, OUT])    # w_proj^T
    bq_a_d = din("bq_a", [E, 1])      # scale * b_q_rgb
    bk_a_d = din("bk_a", [E, 1])      # b_k_pl
    bq_b_d = din("bq_b", [E, 1])      # scale * b_q_pl
    bk_b_d = din("bk_b", [E, 1])      # b_k_rgb
    inv_d = din("bn_inv", [OUT, 1])
    shf_d = din("bn_shf", [OUT, 1])
    ones2_d = din("ones2", [E, 2, 16], mybir.dt.uint8)  # fp8 ones, padded
    ones_r_d = din("ones_r", [1, E])
    ident_d = din("ident", [E, E], mybir.dt.uint16)     # bf16 bits
    y_d = nc.dram_tensor("y", [OUT, n], f32, kind="ExternalOutput").ap()

    with tile.TileContext(nc) as tc:
        # PSUM: st 3x2 banks + ot 1 + dn/bc/yp shared 1 = 8 banks.
        with tc.tile_pool(name="const", bufs=1) as const, \
             tc.tile_pool(name="qkv", bufs=1) as qkv, \
             tc.tile_pool(name="feat", bufs=1) as feat, \
             tc.tile_pool(name="pst", bufs=3, space="PSUM") as pst, \
             tc.tile_pool(name="pot", bufs=1, space="PSUM") as pot, \
             tc.tile_pool(name="pshared", bufs=1, space="PSUM") as pshared, \
             tc.tile_pool(name="sex", bufs=7) as sex, \
             tc.tile_pool(name="sot", bufs=10) as sot, \
             tc.tile_pool(name="sbc", bufs=3) as sbc, \
             tc.tile_pool(name="smisc", bufs=4) as smisc:

            # ---- DMA: dir-a critical path first ----
            def wload(d, nm):
                t = const.tile([128, 2, E], bf16, name=nm, tag=nm)
                nc.sync.dma_start(t[:], d.rearrange("(c p) e -> p c e", p=128).bitcast(bf16))
                return t

            def vload(d, shape, nm, dt_=None, eng=None):
                t = const.tile(shape, dt_ or f32, name=nm, tag=nm)
                (eng or nc.sync).dma_start(t[:], d.bitcast(dt_) if dt_ else d)
                return t

            # dir-a critical path: wk_a (K0), wq_a (Q0) before the features
            wk = {0: wload(wk_a_d, "wka")}
            wq = {0: wload(wq_a_d, "wqa")}
            bk = {0: vload(bk_a_d, [E, 1], "bka")}
            bq = {0: vload(bq_a_d, [E, 1], "bqa")}

            fsb = {
                name: feat.tile([128, 2, n], bf16, tag=f"f{name}",
                                name=f"f_{name}")
                for name in ("a", "b")
            }
            npc = max(1, n // 512)    # 512-col pieces
            # First pieces of BOTH tensors first (K0 needs f_b[0], Q0 needs
            # f_a[0]); rest streams behind on the two HWDGE queues (sync +
            # scalar). gpsimd stays DMA-free so its tail drain is cheap.
            def fpiece(name, pc):
                d_src = f_b_d if name == "b" else f_a_d
                lo, hi = pc * (n // npc), (pc + 1) * (n // npc)
                for cc in range(2):
                    eng = nc.sync if cc == 0 else nc.scalar
                    eng.dma_start(
                        fsb[name][:, cc, lo:hi],
                        d_src[cc * 128:(cc + 1) * 128, lo:hi].bitcast(bf16),
                    )

            fpiece("b", 0)
            fpiece("a", 0)
            wv = {0: wload(wv_a_d, "wva")}
            ones2 = vload(ones2_d, [E, 2, 16], "on2", fp8)
            ones_r = vload(ones_r_d, [1, E], "onr", f32r)
            for pc in range(1, npc):
                fpiece("b", pc)
                fpiece("a", pc)
            # dir-b weights + late consts
            wk[1] = wload(wk_b_d, "wkb")
            wv[1] = wload(wv_b_d, "wvb")
            wq[1] = wload(wq_b_d, "wqb")
            bk[1] = vload(bk_b_d, [E, 1], "bkb")
            bq[1] = vload(bq_b_d, [E, 1], "bqb")
            wp = const.tile([128, 2, OUT], f32r, name="wp", tag="wp")
            nc.sync.dma_start(wp[:], wp_d.rearrange("(c p) e -> p c e", p=128).bitcast(f32r))
            binv = const.tile([128, 2, 1], f32, name="binv", tag="binv")
            nc.sync.dma_start(binv[:], inv_d.rearrange("(c p) e -> p c e", p=128))
            bshf = const.tile([128, 2, 1], f32, name="bshf", tag="bshf")
            nc.sync.dma_start(bshf[:], shf_d.rearrange("(c p) e -> p c e", p=128))

            # ---- per-direction activations ----
            q_sb = {d: qkv.tile([128, n], bf16, tag=f"q{d}", name=f"q_sb{d}") for d in (0, 1)}
            k_sb = {d: qkv.tile([128, n], bf16, tag=f"k{d}", name=f"k_sb{d}") for d in (0, 1)}
            vt_sb = {d: qkv.tile([128, nkc, 128], fp8, tag=f"v{d}", name=f"vt_sb{d}")
                     for d in (0, 1)}
            vtb = {d: qkv.tile([128, nkc, 128], bf16, tag=f"vb{d}", name=f"vtb{d}")
                   for d in (0, 1)}
            vtmps = {d: feat.tile([128, n], bf16, tag=f"vtmp{d}", name=f"vtmp{d}")
                     for d in (0, 1)}

            def emit_proj(d, kind, nt):
                """One 512-wide projection tile: kind in k/v/q."""
                fq = fsb["a"] if d == 0 else fsb["b"]
                fk = fsb["b"] if d == 0 else fsb["a"]
                wt, bias, dst, src = {
                    "k": (wk[d], bk[d], k_sb[d], fk),
                    "v": (wv[d], None, vtmps[d], fk),
                    "q": (wq[d], bq[d], q_sb[d], fq),
                }[kind]
                ps = pst.tile([128, QW], f32, tag="st", name="psp")
                for cc in range(2):
                    nc.tensor.matmul(
                        ps[:],
                        wt[:, cc, :],
                        src[:, cc, nt * QW:(nt + 1) * QW],
                        start=(cc == 0),
                        stop=(cc == 1),
                    )
                with nc.allow_low_precision(reason="bf16 proj"):
                    if bias is None:
                        nc.vector.tensor_copy(
                            dst[:, nt * QW:(nt + 1) * QW], ps[:])
                    else:
                        nc.vector.tensor_scalar_add(
                            dst[:, nt * QW:(nt + 1) * QW], ps[:], bias[:])

            def emit_trans(d, g):
                """Transpose 4 v chunks on the DMA xbar, cast to fp8 on
                gpsimd (both off the compute-critical engines)."""
                for jj in range(4):
                    kc = 4 * g + jj
                    eng = nc.sync if jj % 2 == 0 else nc.scalar
                    eng.dma_start_transpose(
                        vtb[d][:, kc, :],
                        vtmps[d][:, kc * 128:(kc + 1) * 128],
                    )
                with nc.allow_low_precision(reason="fp8 VT"):
                    nc.gpsimd.tensor_copy(
                        vt_sb[d][:, 4 * g:4 * (g + 1), :],
                        vtb[d][:, 4 * g:4 * (g + 1), :],
                    )

            # ---- attention ----
            def emit_S(d, qt, g):
                """S^T matmuls for one k-chunk group -> st psum tile."""
                qs = q_sb[d][:, qt * QW:(qt + 1) * QW]
                st = pst.tile([128, gj, QW], f32, tag="st", name="st")
                for jj in range(gj):
                    j = gj * g + jj
                    nc.tensor.matmul(
                        st[:, jj, :],
                        k_sb[d][:, j * 128:(j + 1) * 128],
                        qs,
                        start=True, stop=True,
                    )
                return st

            def emit_body(d, qt, st_pre, nxt):
                """exp + O/dn accumulation for one (qt, d). st_pre holds the
                2 pre-emitted S tiles (groups 0/1); the loop keeps 2 groups
                of S prefetched, crossing into segment `nxt` at the end so
                the exp engines never drain at segment boundaries.
                Returns (ot, dn, next segment's prefetched S tiles)."""
                ot = pot.tile([128, QW], f32, tag="ot", name="ot")
                dn = pshared.tile([16, QW], f32, tag="sh", name="dn")
                sts = list(st_pre)
                nxt_pre = []
                dn_pending = []   # (ex tile, group, flush-at group)
                ex_hold = None

                def emit_O(g, ext):
                    nc.tensor.matmul(
                        ot[:],
                        vt_sb[d][:, gj * g:gj * (g + 1), :],
                        ext[:],
                        start=(g == 0), stop=(g == ngrp - 1),
                        perf_mode=DR,
                    )

                def emit_dn(ext, gd):
                    nc.tensor.matmul(
                        dn[:], ones2[:], ext[:],
                        start=(gd == 0), stop=(gd == ngrp - 1),
                        perf_mode=DR,
                    )

                for g in range(ngrp):
                    if g + 2 < ngrp:
                        sts.append(emit_S(d, qt, g + 2))
                    elif nxt is not None:
                        nq, nd = nxt
                        nxt_pre.append(emit_S(nd, nq, g + 2 - ngrp))
                    # dn matmuls trail so the in-order PE never waits on exp
                    while dn_pending and dn_pending[0][2] <= g:
                        ext, gd, _ = dn_pending.pop(0)
                        emit_dn(ext, gd)
                    st_cur = sts[g]
                    ex = sex.tile([128, gj, QW], fp8, tag="ex", name="ex")
                    if (g % ngrp) in DVE_GROUPS:
                        with nc.allow_low_precision(reason="fast exp8"):
                            nc.vector.tensor_scalar(
                                ex[:].bitcast(u8), st_cur[:],
                                FE8_A, FE8_B, ALU.mult, ALU.add,
                            )
                    else:
                        nc.scalar.activation(ex[:], st_cur[:], AFT.Exp)
                    # group 0's O-matmul runs at group 1: the fresh ot psum
                    # bank may still be read by the previous segment's
                    # normalize mul, and this grants it a group of slack
                    if g == 0:
                        ex_hold = ex
                    else:
                        if g == 1:
                            emit_O(0, ex_hold)
                        emit_O(g, ex)
                    dn_pending.append((ex, g, g + DN_DELAY))
                for ext, gd, _ in dn_pending:
                    emit_dn(ext, gd)
                return ot, dn, nxt_pre

            def emit_tail(ot, dn):
                """softmax normalize -> osb (f32r SBUF)."""
                rc = smisc.tile([1, QW], f32, tag="rc", name="rc")
                nc.vector.reciprocal_approx_fast(rc[:], dn[0:1, :])
                rcr = smisc.tile([1, QW], f32r, tag="rcr", name="rcr")
                with nc.allow_low_precision(reason="f32r recip"):
                    nc.vector.tensor_copy(rcr[:], rc[:])
                # 1/dn broadcast via Kc=1 matmul
                bc_ps = pshared.tile([128, QW], f32, tag="sh", name="bc")
                nc.tensor.matmul(
                    bc_ps[:], ones_r[:], rcr[:],
                    start=True, stop=True,
                )
                bc_sb = sbc.tile([128, QW], f32r, tag="bcs", name="bcs")
                nc.vector.tensor_copy(bc_sb[:], bc_ps[:])
                osb = sot.tile([128, QW], f32r, tag="osb", name="osb")
                with nc.allow_low_precision(reason="f32r osb"):
                    nc.vector.tensor_mul(osb[:], ot[:], bc_sb[:])
                return osb

            def emit_final(qt, ot_sbs):
                for dch in range(2):
                    yp = pshared.tile([128, QW], f32, tag="sh", name="yp")
                    for d in (0, 1):
                        nc.tensor.matmul(
                            yp[:],
                            wp[:, d, dch * 128:(dch + 1) * 128],
                            ot_sbs[d][:],
                            start=(d == 0), stop=(d == 1),
                        )
                    ysb = smisc.tile([128, QW], f32, tag="ysb", name="ysb")
                    nc.scalar.activation(
                        ysb[:], yp[:], AFT.Relu,
                        bias=bshf[:, dch, :], scale=binv[:, dch, :],
                    )
                    nc.sync.dma_start(
                        y_d[dch * 128:(dch + 1) * 128,
                            qt * QW:(qt + 1) * QW],
                        ysb[:],
                    )

            # ---- schedule ----
            # dir-a K0/Q0 + first S-tile so exp starts as soon as the first
            # feature pieces land; remaining dir-a projections follow; dir-b
            # projection tiles slip between the first dir-a segments in
            # small chunks so PE tail-stall slack absorbs them.
            emit_proj(0, "k", 0)
            emit_proj(0, "q", 0)
            st_next0 = emit_S(0, 0, 0)
            emit_proj(0, "v", 0)
            emit_trans(0, 0)
            for nt in range(1, nqt):
                emit_proj(0, "k", nt)
                emit_proj(0, "v", nt)
                emit_proj(0, "q", nt)
            for nt in range(1, nqt):
                emit_trans(0, nt)

            def fill_k1():
                for nt in range(nqt):
                    emit_proj(1, "k", nt)

            def fill_v1():
                for nt in range(nqt):
                    emit_proj(1, "v", nt)
                for nt in range(nqt):
                    emit_trans(1, nt)

            def fill_q1():
                for nt in range(nqt):
                    emit_proj(1, "q", nt)

            fillers = [fill_k1, fill_v1, fill_q1]
            segs = [(qt, 0) for qt in range(nqt)] + [(qt, 1) for qt in range(nqt)]

            st_pre = [st_next0, emit_S(0, 0, 1)]
            pending = {}          # qt -> {d: osb}
            for i, (qt, d) in enumerate(segs):
                nxt = segs[i + 1] if i + 1 < len(segs) else None
                ot, dn, st_pre = emit_body(d, qt, st_pre, nxt)
                osb = emit_tail(ot, dn)
                pending.setdefault(qt, {})[d] = osb
                # finals run one segment late so the yp matmuls always have
                # a full segment of PE work as cover for the tail chain
                if d == 1 and qt > 0:
                    emit_final(qt - 1, pending.pop(qt - 1))
                if i < len(fillers):
                    fillers[i]()
            emit_final(nqt - 1, pending.pop(nqt - 1))
    nc.compile()
    return nc


def _to_bf16_bits(x):
    u = np.ascontiguousarray(x, np.float32).view(np.uint32)
    r = ((u + 0x7FFF + ((u >> 16) & 1)) >> 16).astype(np.uint16)
    return r


def _to_fp8e4_bits(x):
    import ml_dtypes

    return np.ascontiguousarray(x, np.float32).astype(
        ml_dtypes.float8_e4m3fn).view(np.uint8)


def _host_prep(inputs, n=N):
    f_rgb = _to_bf16_bits(inputs["f_rgb"].reshape(B, C, n))
    f_pl = _to_bf16_bits(inputs["f_pl"].reshape(B, C, n))

    def T(w, scale=1.0):
        return np.ascontiguousarray(scale * np.asarray(w, np.float32).T)

    def T16(w, scale=1.0):
        return _to_bf16_bits(T(w, scale))

    wp = np.asarray(inputs["w_proj"], np.float32)
    inv = np.asarray(inputs["bn_gamma"], np.float32) / np.sqrt(
        np.asarray(inputs["bn_var"], np.float32) + 1e-5)
    shift = (np.asarray(inputs["bn_beta"], np.float32)
             - np.asarray(inputs["bn_mean"], np.float32) * inv
             + inv * (wp[:, :E] @ np.asarray(inputs["b_v_pl"], np.float32)
                      + wp[:, E:] @ np.asarray(inputs["b_v_rgb"], np.float32)))

    shared = {
        "wq_a": T16(inputs["w_q_rgb"], SCALE),
        "wk_a": T16(inputs["w_k_pl"]),
        "wv_a": T16(inputs["w_v_pl"]),
        "wq_b": T16(inputs["w_q_pl"], SCALE),
        "wk_b": T16(inputs["w_k_rgb"]),
        "wv_b": T16(inputs["w_v_rgb"]),
        "wp": T(wp),
        "bq_a": (SCALE * np.asarray(inputs["b_q_rgb"], np.float32))
        .reshape(E, 1).copy(),
        "bk_a": np.asarray(inputs["b_k_pl"], np.float32).reshape(E, 1).copy(),
        "bq_b": (SCALE * np.asarray(inputs["b_q_pl"], np.float32))
        .reshape(E, 1).copy(),
        "bk_b": np.asarray(inputs["b_k_rgb"], np.float32).reshape(E, 1).copy(),
        "bn_inv": inv.reshape(OUT, 1).copy(),
        "bn_shf": shift.reshape(OUT, 1).copy(),
        "ones2": _to_fp8e4_bits(np.ones((E, 2, 16), np.float32)),
        "ones_r": np.ones((1, E), np.float32),
        "ident": _to_bf16_bits(np.eye(E, dtype=np.float32)),
    }
    in_maps = []
    for b in range(B):
        m = dict(shared)
        m["f_a"] = f_rgb[b]
        m["f_b"] = f_pl[b]
        in_maps.append(m)
    return in_maps


def kernel(**inputs):
    from concourse import bass_utils

    if "nc" not in _CACHE:
        _CACHE["nc"] = build_nc()
    nc = _CACHE["nc"]
    in_maps = _host_prep(inputs)
    res = bass_utils.run_bass_kernel_spmd(nc, in_maps, core_ids=list(range(B)))
    out = np.stack([res.results[b]["y"] for b in range(B)], axis=0)
    return out.reshape(B, OUT, H, W).astype(np.float32)


if __name__ == "__main__":
    pass


# revision 5
# speedup vs baseline: 1.1543x; 1.1543x over previous
"""CrossModalAttention TRN2 kernel (v2: fp8 DoubleRow attention).

Strategy (data-parallel over batch, one batch element per NeuronCore):
  dir a: q from rgb, k/v from pl;  dir b: q from pl, k/v from rgb.
  Per direction:
    Q  = scale*(Wq @ f_q + bq)        [128 e, N] bf16 (scale folded into W,b)
    K  = Wk @ f_k + bk                [128 e, N] bf16
    VT = (Wv @ f_k)^T                 [N k, 128 e] fp8e4m3 (v-bias folded
                                      into the BN shift host-side)
    per q-tile (512 wide), per group g of 2 k-chunks:
      S^T_g = K_g^T @ Q_tile          [128 k, 2, 512 q]  (PSUM f32)
      E_g   = exp(S^T_g) -> fp8       ScalarE for most groups; VectorE
                                      computes e4m3 bits directly via the
                                      round(x*8*log2e + 55.5) affine trick
                                      for DVE_GROUPS (engine balance)
      OT   += VT_g^T @ E_g            one fp8 DoubleRow matmul (256-row
                                      contraction, 2x col rate)
      dn   += ones^T @ E_g            one fp8 DoubleRow matmul (weight padded
                                      to 16 cols for the lw step%16 rule),
                                      delayed 3 groups to stay off the
                                      critical path
      OT_norm = OT * bcast(1/dn)      reciprocal_approx_fast on DVE; bcast
                                      via Kc=1 rank-1 matmul
  y = Wp_a @ OT_a + Wp_b @ OT_b ; out = relu(inv*y + shift)  (BN folded)

Schedule: dir-a K/V features DMA first; dir-a projections then dir-a
attention start immediately, with dir-b projections slipped between the
first dir-a segments so the exp engines start ~40us earlier than a
proj-everything-first order.
"""

import sys

sys.path.insert(0, "/opt/trn_rl_repo")

import numpy as np

B = 8
C = 256
E = 128
OUT = 256
H = W = 64
N = H * W
QW = 512
SCALE = float(E) ** -0.5

LOG2E = 1.4426950408889634
FE8_A = 8.0 * LOG2E          # e4m3 bits = round(s*FE8_A + FE8_B)
FE8_B = 7.0 * 8.0 - 0.5      # HW float->uint8 rounds to nearest; c=-0.5
# groups (of 16 per segment) whose exp runs on DVE instead of ScalarE
DVE_GROUPS = frozenset({5, 10, 14})

_CACHE = {}


def _patch_tail_drain(tile_mod, mybir):
    # This walrus build encodes Drain as CTRL_NO_STRUCT with a single
    # sync-wait slot; split the TileContext tail drain's waits across
    # one drain instruction per semaphore.
    if getattr(tile_mod.TileContext, "_drain_patched", False):
        return
    from concourse.vector_clock import ScopedClock

    def _drain_and_barrier(self, tick_clock, wait_clock):
        nc = self.nc
        drain_inst = nc.sync.drain()
        wait_clock.add_sem_waits(
            drain_inst.ins, ScopedClock({None: tick_clock.global_clock})
        )
        si = drain_inst.ins.sync_info
        if si is not None and si.on_wait and len(si.on_wait) > 1:
            waits = list(si.on_wait)
            drain_inst.ins.sync_info = mybir.SyncInfo(
                on_wait=[waits[0]], on_update=list(si.on_update or [])
            )
            for w in waits[1:]:
                d2 = nc.sync.drain()
                d2.ins.sync_info = mybir.SyncInfo(on_wait=[w], on_update=[])
        nc.all_engine_barrier()
        popped = nc._tile_sem_poison_stack.pop()
        assert popped is self._sem_poison
        nc.clear_and_free_semaphores(list(self.sems.allocated().values()))
        nc.all_engine_barrier()

    tile_mod.TileContext._drain_and_barrier = _drain_and_barrier
    tile_mod.TileContext._drain_patched = True


def build_nc(n=N, debug=False):
    """Build the single-core Bass program. n = spatial size (4096 full)."""
    import concourse.bacc as bacc
    import concourse.tile as tile
    from concourse import mybir

    f32 = mybir.dt.float32
    f32r = mybir.dt.float32r
    bf16 = mybir.dt.bfloat16
    fp8 = mybir.dt.float8e4
    u8 = mybir.dt.uint8
    AFT = mybir.ActivationFunctionType
    ALU = mybir.AluOpType
    DR = mybir.MatmulPerfMode.DoubleRow

    gj = 2                  # k-chunks per PSUM S-tile / exp instruction
    nqt = n // QW
    nkc = n // 128
    ngrp = nkc // gj        # exp groups per segment
    DN_DELAY = 3            # groups the dn matmul trails its exp by

    nc = bacc.Bacc(trn_type="TRN2", target_bir_lowering=False, debug=False)

    def din(name, shape, dt_=f32):
        return nc.dram_tensor(name, shape, dt_, kind="ExternalInput").ap()

    u16 = mybir.dt.uint16
    f_a_d = din("f_a", [C, n], u16)   # rgb features bf16 bits (q-side of a)
    f_b_d = din("f_b", [C, n], u16)   # pl features bf16 bits
    wq_a_d = din("wq_a", [C, E], u16)  # scale * W_q_rgb^T (bf16 bits)
    wk_a_d = din("wk_a", [C, E], u16)  # W_k_pl^T
    wv_a_d = din("wv_a", [C, E], u16)  # W_v_pl^T
    wq_b_d = din("wq_b", [C, E], u16)  # scale * W_q_pl^T
    wk_b_d = din("wk_b", [C, E], u16)  # W_k_rgb^T
    wv_b_d = din("wv_b", [C, E], u16)  # W_v_rgb^T
    wp_d = din("wp", [2 * E, OUT])    # w_proj^T
    bq_a_d = din("bq_a", [E, 1])      # scale * b_q_rgb
    bk_a_d = din("bk_a", [E, 1])      # b_k_pl
    bq_b_d = din("bq_b", [E, 1])      # scale * b_q_pl
    bk_b_d = din("bk_b", [E, 1])      # b_k_rgb
    inv_d = din("bn_inv", [OUT, 1])
    shf_d = din("bn_shf", [OUT, 1])
    ones2_d = din("ones2", [E, 2, 16], mybir.dt.uint8)  # fp8 ones, padded
    ones_r_d = din("ones_r", [1, E])
    ident_d = din("ident", [E, E], mybir.dt.uint16)     # bf16 bits
    y_d = nc.dram_tensor("y", [OUT, n], f32, kind="ExternalOutput").ap()

    with tile.TileContext(nc) as tc:
        # PSUM: st 3x2 banks + ot 1 + dn/bc/yp shared 1 = 8 banks.
        with tc.tile_pool(name="const", bufs=1) as const, \
             tc.tile_pool(name="qkv", bufs=1) as qkv, \
             tc.tile_pool(name="feat", bufs=1) as feat, \
             tc.tile_pool(name="pst", bufs=3, space="PSUM") as pst, \
             tc.tile_pool(name="pot", bufs=1, space="PSUM") as pot, \
             tc.tile_pool(name="pshared", bufs=1, space="PSUM") as pshared, \
             tc.tile_pool(name="sex", bufs=7) as sex, \
             tc.tile_pool(name="sot", bufs=10) as sot, \
             tc.tile_pool(name="sbc", bufs=3) as sbc, \
             tc.tile_pool(name="smisc", bufs=4) as smisc:

            # ---- DMA: dir-a critical path first ----
            def wload(d, nm):
                t = const.tile([128, 2, E], bf16, name=nm, tag=nm)
                nc.sync.dma_start(t[:], d.rearrange("(c p) e -> p c e", p=128).bitcast(bf16))
                return t

            def vload(d, shape, nm, dt_=None, eng=None):
                t = const.tile(shape, dt_ or f32, name=nm, tag=nm)
                (eng or nc.sync).dma_start(t[:], d.bitcast(dt_) if dt_ else d)
                return t

            # dir-a critical path: wk_a (K0), wq_a (Q0) before the features
            wk = {0: wload(wk_a_d, "wka")}
            wq = {0: wload(wq_a_d, "wqa")}
            bk = {0: vload(bk_a_d, [E, 1], "bka")}
            bq = {0: vload(bq_a_d, [E, 1], "bqa")}

            fsb = {
                name: feat.tile([128, 2, n], bf16, tag=f"f{name}",
                                name=f"f_{name}")
                for name in ("a", "b")
            }
            npc = max(1, n // 512)    # 512-col pieces
            # First pieces of BOTH tensors first (K0 needs f_b[0], Q0 needs
            # f_a[0]); rest streams behind on the two HWDGE queues (sync +
            # scalar). gpsimd stays DMA-free so its tail drain is cheap.
            def fpiece(name, pc):
                d_src = f_b_d if name == "b" else f_a_d
                lo, hi = pc * (n // npc), (pc + 1) * (n // npc)
                for cc in range(2):
                    eng = nc.sync if cc == 0 else nc.scalar
                    eng.dma_start(
                        fsb[name][:, cc, lo:hi],
                        d_src[cc * 128:(cc + 1) * 128, lo:hi].bitcast(bf16),
                    )

            fpiece("b", 0)
            fpiece("a", 0)
            wv = {0: wload(wv_a_d, "wva")}
            ident = vload(ident_d, [E, E], "idt", bf16)
            ones2 = vload(ones2_d, [E, 2, 16], "on2", fp8)
            ones_r = vload(ones_r_d, [1, E], "onr", f32r)
            for pc in range(1, npc):
                fpiece("b", pc)
                fpiece("a", pc)
            # dir-b weights + late consts
            wk[1] = wload(wk_b_d, "wkb")
            wv[1] = wload(wv_b_d, "wvb")
            wq[1] = wload(wq_b_d, "wqb")
            bk[1] = vload(bk_b_d, [E, 1], "bkb")
            bq[1] = vload(bq_b_d, [E, 1], "bqb")
            wp = const.tile([128, 2, OUT], f32r, name="wp", tag="wp")
            nc.sync.dma_start(wp[:], wp_d.rearrange("(c p) e -> p c e", p=128).bitcast(f32r))
            binv = const.tile([128, 2, 1], f32, name="binv", tag="binv")
            nc.sync.dma_start(binv[:], inv_d.rearrange("(c p) e -> p c e", p=128))
            bshf = const.tile([128, 2, 1], f32, name="bshf", tag="bshf")
            nc.sync.dma_start(bshf[:], shf_d.rearrange("(c p) e -> p c e", p=128))

            # ---- per-direction activations ----
            q_sb = {d: qkv.tile([128, n], bf16, tag=f"q{d}", name=f"q_sb{d}") for d in (0, 1)}
            k_sb = {d: qkv.tile([128, n], bf16, tag=f"k{d}", name=f"k_sb{d}") for d in (0, 1)}
            vt_sb = {d: qkv.tile([128, nkc, 128], fp8, tag=f"v{d}", name=f"vt_sb{d}")
                     for d in (0, 1)}
            vtb = {d: qkv.tile([128, nkc, 128], bf16, tag=f"vb{d}", name=f"vtb{d}")
                   for d in (0, 1)}
            vtmps = {d: feat.tile([128, n], bf16, tag=f"vtmp{d}", name=f"vtmp{d}")
                     for d in (0, 1)}

            def emit_proj(d, kind, nt):
                """One 512-wide projection tile: kind in k/v/q."""
                fq = fsb["a"] if d == 0 else fsb["b"]
                fk = fsb["b"] if d == 0 else fsb["a"]
                wt, bias, dst, src = {
                    "k": (wk[d], bk[d], k_sb[d], fk),
                    "v": (wv[d], None, vtmps[d], fk),
                    "q": (wq[d], bq[d], q_sb[d], fq),
                }[kind]
                ps = pst.tile([128, QW], f32, tag="st", name="psp")
                for cc in range(2):
                    nc.tensor.matmul(
                        ps[:],
                        wt[:, cc, :],
                        src[:, cc, nt * QW:(nt + 1) * QW],
                        start=(cc == 0),
                        stop=(cc == 1),
                    )
                with nc.allow_low_precision(reason="bf16 proj"):
                    if bias is None:
                        nc.vector.tensor_copy(
                            dst[:, nt * QW:(nt + 1) * QW], ps[:])
                    else:
                        nc.vector.tensor_scalar_add(
                            dst[:, nt * QW:(nt + 1) * QW], ps[:], bias[:])

            def emit_trans(d, g):
                """Transpose 4 v chunks -> vt fp8. dir a: PE transpose (low
                latency, feeds the very first segments). dir b: DMA xbar on
                the idle sync queue + gpsimd cast (off the PE; its segments
                start >100us later). Never touch the scalar queue: DMA
                dispatch there stalls the exp stream."""
                if d == 0:
                    ps = pst.tile([128, QW], bf16, tag="st", name="psvt")
                    for jj in range(4):
                        kc = 4 * g + jj
                        nc.tensor.transpose(
                            ps[:, jj * 128:(jj + 1) * 128],
                            vtmps[d][:, kc * 128:(kc + 1) * 128],
                            ident[:],
                        )
                    with nc.allow_low_precision(reason="fp8 VT"):
                        nc.vector.tensor_copy(
                            vt_sb[d][:, 4 * g:4 * (g + 1), :], ps[:]
                        )
                else:
                    for jj in range(4):
                        kc = 4 * g + jj
                        nc.sync.dma_start_transpose(
                            vtb[d][:, kc, :],
                            vtmps[d][:, kc * 128:(kc + 1) * 128],
                        )
                    with nc.allow_low_precision(reason="fp8 VT"):
                        nc.gpsimd.tensor_copy(
                            vt_sb[d][:, 4 * g:4 * (g + 1), :],
                            vtb[d][:, 4 * g:4 * (g + 1), :],
                        )

            # ---- attention ----
            def emit_S(d, qt, g):
                """S^T matmuls for one k-chunk group -> st psum tile."""
                qs = q_sb[d][:, qt * QW:(qt + 1) * QW]
                st = pst.tile([128, gj, QW], f32, tag="st", name="st")
                for jj in range(gj):
                    j = gj * g + jj
                    nc.tensor.matmul(
                        st[:, jj, :],
                        k_sb[d][:, j * 128:(j + 1) * 128],
                        qs,
                        start=True, stop=True,
                    )
                return st

            def emit_body(d, qt, st_pre, nxt):
                """exp + O/dn accumulation for one (qt, d). st_pre holds the
                2 pre-emitted S tiles (groups 0/1); the loop keeps 2 groups
                of S prefetched, crossing into segment `nxt` at the end so
                the exp engines never drain at segment boundaries.
                Returns (ot, dn, next segment's prefetched S tiles)."""
                ot = pot.tile([128, QW], f32, tag="ot", name="ot")
                dn = pshared.tile([16, QW], f32, tag="sh", name="dn")
                sts = list(st_pre)
                nxt_pre = []
                dn_pending = []   # (ex tile, group, flush-at group)
                ex_hold = []

                def emit_O(g, ext):
                    nc.tensor.matmul(
                        ot[:],
                        vt_sb[d][:, gj * g:gj * (g + 1), :],
                        ext[:],
                        start=(g == 0), stop=(g == ngrp - 1),
                        perf_mode=DR,
                    )

                def emit_dn(ext, gd):
                    nc.tensor.matmul(
                        dn[:], ones2[:], ext[:],
                        start=(gd == 0), stop=(gd == ngrp - 1),
                        perf_mode=DR,
                    )

                for g in range(ngrp):
                    if g + 2 < ngrp:
                        sts.append(emit_S(d, qt, g + 2))
                    elif nxt is not None:
                        nq, nd = nxt
                        nxt_pre.append(emit_S(nd, nq, g + 2 - ngrp))
                    # dn matmuls trail so the in-order PE never waits on exp
                    while dn_pending and dn_pending[0][2] <= g:
                        ext, gd, _ = dn_pending.pop(0)
                        emit_dn(ext, gd)
                    st_cur = sts[g]
                    ex = sex.tile([128, gj, QW], fp8, tag="ex", name="ex")
                    if (g % ngrp) in DVE_GROUPS:
                        with nc.allow_low_precision(reason="fast exp8"):
                            nc.vector.tensor_scalar(
                                ex[:].bitcast(u8), st_cur[:],
                                FE8_A, FE8_B, ALU.mult, ALU.add,
                            )
                    else:
                        nc.scalar.activation(ex[:], st_cur[:], AFT.Exp)
                    # the first 2 O-matmuls run at group 2: the fresh ot
                    # psum bank may still be read by the previous segment's
                    # normalize mul, and this grants 2 groups of slack
                    if g < 2:
                        ex_hold.append(ex)
                    else:
                        if g == 2:
                            for gh, exh in enumerate(ex_hold):
                                emit_O(gh, exh)
                        emit_O(g, ex)
                    dn_pending.append((ex, g, g + DN_DELAY))
                for ext, gd, _ in dn_pending:
                    emit_dn(ext, gd)
                return ot, dn, nxt_pre

            def emit_tail(ot, dn):
                """softmax normalize -> osb (f32r SBUF)."""
                rc = smisc.tile([1, QW], f32, tag="rc", name="rc")
                nc.vector.reciprocal_approx_fast(rc[:], dn[0:1, :])
                rcr = smisc.tile([1, QW], f32r, tag="rcr", name="rcr")
                with nc.allow_low_precision(reason="f32r recip"):
                    nc.vector.tensor_copy(rcr[:], rc[:])
                # 1/dn broadcast via Kc=1 matmul
                bc_ps = pshared.tile([128, QW], f32, tag="sh", name="bc")
                nc.tensor.matmul(
                    bc_ps[:], ones_r[:], rcr[:],
                    start=True, stop=True,
                )
                bc_sb = sbc.tile([128, QW], f32r, tag="bcs", name="bcs")
                nc.vector.tensor_copy(bc_sb[:], bc_ps[:])
                osb = sot.tile([128, QW], f32r, tag="osb", name="osb")
                with nc.allow_low_precision(reason="f32r osb"):
                    nc.vector.tensor_mul(osb[:], ot[:], bc_sb[:])
                return osb

            def emit_final(qt, ot_sbs):
                for dch in range(2):
                    yp = pshared.tile([128, QW], f32, tag="sh", name="yp")
                    for d in (0, 1):
                        nc.tensor.matmul(
                            yp[:],
                            wp[:, d, dch * 128:(dch + 1) * 128],
                            ot_sbs[d][:],
                            start=(d == 0), stop=(d == 1),
                        )
                    ysb = smisc.tile([128, QW], f32, tag="ysb", name="ysb")
                    nc.scalar.activation(
                        ysb[:], yp[:], AFT.Relu,
                        bias=bshf[:, dch, :], scale=binv[:, dch, :],
                    )
                    nc.sync.dma_start(
                        y_d[dch * 128:(dch + 1) * 128,
                            qt * QW:(qt + 1) * QW],
                        ysb[:],
                    )

            # ---- schedule ----
            # dir-a K0/Q0 + first S-tile so exp starts as soon as the first
            # feature pieces land; remaining dir-a projections follow; dir-b
            # projection tiles slip between the first dir-a segments in
            # small chunks so PE tail-stall slack absorbs them.
            emit_proj(0, "k", 0)
            emit_proj(0, "q", 0)
            st_next0 = emit_S(0, 0, 0)
            emit_proj(0, "v", 0)
            emit_trans(0, 0)
            for nt in range(1, nqt):
                emit_proj(0, "k", nt)
                emit_proj(0, "v", nt)
                emit_proj(0, "q", nt)
            for nt in range(1, nqt):
                emit_trans(0, nt)

            def fill_k1():
                for nt in range(nqt):
                    emit_proj(1, "k", nt)

            def fill_v1():
                for nt in range(nqt):
                    emit_proj(1, "v", nt)
                for nt in range(nqt):
                    emit_trans(1, nt)

            def fill_q1():
                for nt in range(nqt):
                    emit_proj(1, "q", nt)

            fillers = [fill_k1, fill_v1, fill_q1]
            segs = [(qt, 0) for qt in range(nqt)] + [(qt, 1) for qt in range(nqt)]

            st_pre = [st_next0, emit_S(0, 0, 1)]
            pending = {}          # qt -> {d: osb}
            for i, (qt, d) in enumerate(segs):
                nxt = segs[i + 1] if i + 1 < len(segs) else None
                ot, dn, st_pre = emit_body(d, qt, st_pre, nxt)
                osb = emit_tail(ot, dn)
                pending.setdefault(qt, {})[d] = osb
                # finals run one segment late so the yp matmuls always have
                # a full segment of PE work as cover for the tail chain
                if d == 1 and qt > 0:
                    emit_final(qt - 1, pending.pop(qt - 1))
                if i < len(fillers):
                    fillers[i]()
            emit_final(nqt - 1, pending.pop(nqt - 1))
    nc.compile()
    return nc


def _to_bf16_bits(x):
    u = np.ascontiguousarray(x, np.float32).view(np.uint32)
    r = ((u + 0x7FFF + ((u >> 16) & 1)) >> 16).astype(np.uint16)
    return r


def _to_fp8e4_bits(x):
    import ml_dtypes

    return np.ascontiguousarray(x, np.float32).astype(
        ml_dtypes.float8_e4m3fn).view(np.uint8)


def _host_prep(inputs, n=N):
    f_rgb = _to_bf16_bits(inputs["f_rgb"].reshape(B, C, n))
    f_pl = _to_bf16_bits(inputs["f_pl"].reshape(B, C, n))

    def T(w, scale=1.0):
        return np.ascontiguousarray(scale * np.asarray(w, np.float32).T)

    def T16(w, scale=1.0):
        return _to_bf16_bits(T(w, scale))

    wp = np.asarray(inputs["w_proj"], np.float32)
    inv = np.asarray(inputs["bn_gamma"], np.float32) / np.sqrt(
        np.asarray(inputs["bn_var"], np.float32) + 1e-5)
    shift = (np.asarray(inputs["bn_beta"], np.float32)
             - np.asarray(inputs["bn_mean"], np.float32) * inv
             + inv * (wp[:, :E] @ np.asarray(inputs["b_v_pl"], np.float32)
                      + wp[:, E:] @ np.asarray(inputs["b_v_rgb"], np.float32)))

    shared = {
        "wq_a": T16(inputs["w_q_rgb"], SCALE),
        "wk_a": T16(inputs["w_k_pl"]),
        "wv_a": T16(inputs["w_v_pl"]),
        "wq_b": T16(inputs["w_q_pl"], SCALE),
        "wk_b": T16(inputs["w_k_rgb"]),
        "wv_b": T16(inputs["w_v_rgb"]),
        "wp": T(wp),
        "bq_a": (SCALE * np.asarray(inputs["b_q_rgb"], np.float32))
        .reshape(E, 1).copy(),
        "bk_a": np.asarray(inputs["b_k_pl"], np.float32).reshape(E, 1).copy(),
        "bq_b": (SCALE * np.asarray(inputs["b_q_pl"], np.float32))
        .reshape(E, 1).copy(),
        "bk_b": np.asarray(inputs["b_k_rgb"], np.float32).reshape(E, 1).copy(),
        "bn_inv": inv.reshape(OUT, 1).copy(),
        "bn_shf": shift.reshape(OUT, 1).copy(),
        "ones2": _to_fp8e4_bits(np.ones((E, 2, 16), np.float32)),
        "ones_r": np.ones((1, E), np.float32),
        "ident": _to_bf16_bits(np.eye(E, dtype=np.float32)),
    }
    in_maps = []
    for b in range(B):
        m = dict(shared)
        m["f_a"] = f_rgb[b]
        m["f_b"] = f_pl[b]
        in_maps.append(m)
    return in_maps


def kernel(**inputs):
    from concourse import bass_utils

    if "nc" not in _CACHE:
        _CACHE["nc"] = build_nc()
    nc = _CACHE["nc"]
    in_maps = _host_prep(inputs)
    res = bass_utils.run_bass_kernel_spmd(nc, in_maps, core_ids=list(range(B)))
    out = np.stack([res.results[b]["y"] for b in range(B)], axis=0)
    return out.reshape(B, OUT, H, W).astype(np.float32)


if __name__ == "__main__":
    pass


# revision 6
# speedup vs baseline: 1.1648x; 1.0091x over previous
"""CrossModalAttention TRN2 kernel (v2: fp8 DoubleRow attention).

Strategy (data-parallel over batch, one batch element per NeuronCore):
  dir a: q from rgb, k/v from pl;  dir b: q from pl, k/v from rgb.
  Per direction:
    Q  = scale*(Wq @ f_q + bq)        [128 e, N] bf16 (scale folded into W,b)
    K  = Wk @ f_k + bk                [128 e, N] bf16
    VT = (Wv @ f_k)^T                 [N k, 128 e] fp8e4m3 (v-bias folded
                                      into the BN shift host-side)
    per q-tile (512 wide), per group g of 2 k-chunks:
      S^T_g = K_g^T @ Q_tile          [128 k, 2, 512 q]  (PSUM f32)
      E_g   = exp(S^T_g) -> fp8       ScalarE for most groups; VectorE
                                      computes e4m3 bits directly via the
                                      round(x*8*log2e + 55.5) affine trick
                                      for DVE_GROUPS (engine balance)
      OT   += VT_g^T @ E_g            one fp8 DoubleRow matmul (256-row
                                      contraction, 2x col rate)
      dn   += ones^T @ E_g            one fp8 DoubleRow matmul (weight padded
                                      to 16 cols for the lw step%16 rule),
                                      delayed 3 groups to stay off the
                                      critical path
      OT_norm = OT * bcast(1/dn)      reciprocal_approx_fast on DVE; bcast
                                      via Kc=1 rank-1 matmul
  y = Wp_a @ OT_a + Wp_b @ OT_b ; out = relu(inv*y + shift)  (BN folded)

Schedule: dir-a K/V features DMA first; dir-a projections then dir-a
attention start immediately, with dir-b projections slipped between the
first dir-a segments so the exp engines start ~40us earlier than a
proj-everything-first order.
"""

import sys

sys.path.insert(0, "/opt/trn_rl_repo")

import numpy as np

B = 8
C = 256
E = 128
OUT = 256
H = W = 64
N = H * W
QW = 512
SCALE = float(E) ** -0.5

LOG2E = 1.4426950408889634
FE8_A = 8.0 * LOG2E          # e4m3 bits = round(s*FE8_A + FE8_B)
FE8_B = 7.0 * 8.0 - 0.5      # HW float->uint8 rounds to nearest; c=-0.5
# groups (of 16 per segment) whose exp runs on DVE instead of ScalarE
DVE_GROUPS = frozenset({4, 8, 12, 15})

_CACHE = {}


def _patch_tail_drain(tile_mod, mybir):
    # This walrus build encodes Drain as CTRL_NO_STRUCT with a single
    # sync-wait slot; split the TileContext tail drain's waits across
    # one drain instruction per semaphore.
    if getattr(tile_mod.TileContext, "_drain_patched", False):
        return
    from concourse.vector_clock import ScopedClock

    def _drain_and_barrier(self, tick_clock, wait_clock):
        nc = self.nc
        drain_inst = nc.sync.drain()
        wait_clock.add_sem_waits(
            drain_inst.ins, ScopedClock({None: tick_clock.global_clock})
        )
        si = drain_inst.ins.sync_info
        if si is not None and si.on_wait and len(si.on_wait) > 1:
            waits = list(si.on_wait)
            drain_inst.ins.sync_info = mybir.SyncInfo(
                on_wait=[waits[0]], on_update=list(si.on_update or [])
            )
            for w in waits[1:]:
                d2 = nc.sync.drain()
                d2.ins.sync_info = mybir.SyncInfo(on_wait=[w], on_update=[])
        nc.all_engine_barrier()
        popped = nc._tile_sem_poison_stack.pop()
        assert popped is self._sem_poison
        nc.clear_and_free_semaphores(list(self.sems.allocated().values()))
        nc.all_engine_barrier()

    tile_mod.TileContext._drain_and_barrier = _drain_and_barrier
    tile_mod.TileContext._drain_patched = True


def build_nc(n=N, debug=False):
    """Build the single-core Bass program. n = spatial size (4096 full)."""
    import concourse.bacc as bacc
    import concourse.tile as tile
    from concourse import mybir

    f32 = mybir.dt.float32
    f32r = mybir.dt.float32r
    bf16 = mybir.dt.bfloat16
    fp8 = mybir.dt.float8e4
    u8 = mybir.dt.uint8
    AFT = mybir.ActivationFunctionType
    ALU = mybir.AluOpType
    DR = mybir.MatmulPerfMode.DoubleRow

    gj = 2                  # k-chunks per PSUM S-tile / exp instruction
    nqt = n // QW
    nkc = n // 128
    ngrp = nkc // gj        # exp groups per segment
    DN_DELAY = 0            # dn consumes ex right after O (no chain)

    nc = bacc.Bacc(trn_type="TRN2", target_bir_lowering=False, debug=False)

    def din(name, shape, dt_=f32):
        return nc.dram_tensor(name, shape, dt_, kind="ExternalInput").ap()

    u16 = mybir.dt.uint16
    f_a_d = din("f_a", [C, n], u16)   # rgb features bf16 bits (q-side of a)
    f_b_d = din("f_b", [C, n], u16)   # pl features bf16 bits
    wq_a_d = din("wq_a", [C, E], u16)  # scale * W_q_rgb^T (bf16 bits)
    wk_a_d = din("wk_a", [C, E], u16)  # W_k_pl^T
    wv_a_d = din("wv_a", [C, E], u16)  # W_v_pl^T
    wq_b_d = din("wq_b", [C, E], u16)  # scale * W_q_pl^T
    wk_b_d = din("wk_b", [C, E], u16)  # W_k_rgb^T
    wv_b_d = din("wv_b", [C, E], u16)  # W_v_rgb^T
    wp_d = din("wp", [2 * E, OUT])    # w_proj^T
    bq_a_d = din("bq_a", [E, 1])      # scale * b_q_rgb
    bk_a_d = din("bk_a", [E, 1])      # b_k_pl
    bq_b_d = din("bq_b", [E, 1])      # scale * b_q_pl
    bk_b_d = din("bk_b", [E, 1])      # b_k_rgb
    inv_d = din("bn_inv", [OUT, 1])
    shf_d = din("bn_shf", [OUT, 1])
    ones2_d = din("ones2", [E, 2, 16], mybir.dt.uint8)  # fp8 ones, padded
    ones_r_d = din("ones_r", [1, E])
    ident_d = din("ident", [E, E], mybir.dt.uint16)     # bf16 bits
    y_d = nc.dram_tensor("y", [OUT, n], f32, kind="ExternalOutput").ap()

    with tile.TileContext(nc) as tc:
        # PSUM: st 3x2 banks + ot 1 + dn/bc/yp shared 1 = 8 banks.
        with tc.tile_pool(name="const", bufs=1) as const, \
             tc.tile_pool(name="qkv", bufs=1) as qkv, \
             tc.tile_pool(name="feat", bufs=1) as feat, \
             tc.tile_pool(name="pst", bufs=3, space="PSUM") as pst, \
             tc.tile_pool(name="pot", bufs=1, space="PSUM") as pot, \
             tc.tile_pool(name="pshared", bufs=1, space="PSUM") as pshared, \
             tc.tile_pool(name="sex", bufs=7) as sex, \
             tc.tile_pool(name="sot", bufs=10) as sot, \
             tc.tile_pool(name="sbc", bufs=3) as sbc, \
             tc.tile_pool(name="smisc", bufs=4) as smisc:

            # ---- DMA: dir-a critical path first ----
            def wload(d, nm):
                t = const.tile([128, 2, E], bf16, name=nm, tag=nm)
                nc.sync.dma_start(t[:], d.rearrange("(c p) e -> p c e", p=128).bitcast(bf16))
                return t

            def vload(d, shape, nm, dt_=None, eng=None):
                t = const.tile(shape, dt_ or f32, name=nm, tag=nm)
                (eng or nc.sync).dma_start(t[:], d.bitcast(dt_) if dt_ else d)
                return t

            # dir-a critical path: wk_a (K0), wq_a (Q0) before the features
            wk = {0: wload(wk_a_d, "wka")}
            wq = {0: wload(wq_a_d, "wqa")}
            bk = {0: vload(bk_a_d, [E, 1], "bka")}
            bq = {0: vload(bq_a_d, [E, 1], "bqa")}

            fsb = {
                name: feat.tile([128, 2, n], bf16, tag=f"f{name}",
                                name=f"f_{name}")
                for name in ("a", "b")
            }
            npc = max(1, n // 512)    # 512-col pieces
            # First pieces of BOTH tensors first (K0 needs f_b[0], Q0 needs
            # f_a[0]); rest streams behind on the two HWDGE queues (sync +
            # scalar). gpsimd stays DMA-free so its tail drain is cheap.
            def fpiece(name, pc):
                d_src = f_b_d if name == "b" else f_a_d
                lo, hi = pc * (n // npc), (pc + 1) * (n // npc)
                for cc in range(2):
                    eng = nc.sync if cc == 0 else nc.scalar
                    eng.dma_start(
                        fsb[name][:, cc, lo:hi],
                        d_src[cc * 128:(cc + 1) * 128, lo:hi].bitcast(bf16),
                    )

            fpiece("b", 0)
            fpiece("a", 0)
            fpiece("b", 1)
            fpiece("a", 1)
            wv = {0: wload(wv_a_d, "wva")}
            ident = vload(ident_d, [E, E], "idt", bf16)
            ones2 = vload(ones2_d, [E, 2, 16], "on2", fp8)
            ones_r = vload(ones_r_d, [1, E], "onr", f32r)
            for pc in range(2, npc):
                fpiece("b", pc)
                fpiece("a", pc)
            # dir-b weights + late consts
            wk[1] = wload(wk_b_d, "wkb")
            wv[1] = wload(wv_b_d, "wvb")
            wq[1] = wload(wq_b_d, "wqb")
            bk[1] = vload(bk_b_d, [E, 1], "bkb")
            bq[1] = vload(bq_b_d, [E, 1], "bqb")
            wp = const.tile([128, 2, OUT], f32r, name="wp", tag="wp")
            nc.sync.dma_start(wp[:], wp_d.rearrange("(c p) e -> p c e", p=128).bitcast(f32r))
            binv = const.tile([128, 2, 1], f32, name="binv", tag="binv")
            nc.sync.dma_start(binv[:], inv_d.rearrange("(c p) e -> p c e", p=128))
            bshf = const.tile([128, 2, 1], f32, name="bshf", tag="bshf")
            nc.sync.dma_start(bshf[:], shf_d.rearrange("(c p) e -> p c e", p=128))

            # ---- per-direction activations ----
            q_sb = {d: qkv.tile([128, n], bf16, tag=f"q{d}", name=f"q_sb{d}") for d in (0, 1)}
            k_sb = {d: qkv.tile([128, n], bf16, tag=f"k{d}", name=f"k_sb{d}") for d in (0, 1)}
            vt_sb = {d: qkv.tile([128, nkc, 128], fp8, tag=f"v{d}", name=f"vt_sb{d}")
                     for d in (0, 1)}
            vtb = {d: qkv.tile([128, nkc, 128], bf16, tag=f"vb{d}", name=f"vtb{d}")
                   for d in (0, 1)}
            vtmps = {d: feat.tile([128, n], bf16, tag=f"vtmp{d}", name=f"vtmp{d}")
                     for d in (0, 1)}

            def emit_proj(d, kind, nt):
                """One 512-wide projection tile: kind in k/v/q."""
                fq = fsb["a"] if d == 0 else fsb["b"]
                fk = fsb["b"] if d == 0 else fsb["a"]
                wt, bias, dst, src = {
                    "k": (wk[d], bk[d], k_sb[d], fk),
                    "v": (wv[d], None, vtmps[d], fk),
                    "q": (wq[d], bq[d], q_sb[d], fq),
                }[kind]
                ps = pst.tile([128, QW], f32, tag="st", name="psp")
                for cc in range(2):
                    nc.tensor.matmul(
                        ps[:],
                        wt[:, cc, :],
                        src[:, cc, nt * QW:(nt + 1) * QW],
                        start=(cc == 0),
                        stop=(cc == 1),
                    )
                with nc.allow_low_precision(reason="bf16 proj"):
                    if bias is None:
                        nc.vector.tensor_copy(
                            dst[:, nt * QW:(nt + 1) * QW], ps[:])
                    else:
                        nc.vector.tensor_scalar_add(
                            dst[:, nt * QW:(nt + 1) * QW], ps[:], bias[:])

            def emit_trans(d, g):
                """Transpose 4 v chunks -> vt fp8. dir a: PE transpose (low
                latency, feeds the very first segments). dir b: DMA xbar on
                the idle sync queue + gpsimd cast (off the PE; its segments
                start >100us later). Never touch the scalar queue: DMA
                dispatch there stalls the exp stream."""
                if d == 0:
                    ps = pst.tile([128, QW], bf16, tag="st", name="psvt")
                    for jj in range(4):
                        kc = 4 * g + jj
                        nc.tensor.transpose(
                            ps[:, jj * 128:(jj + 1) * 128],
                            vtmps[d][:, kc * 128:(kc + 1) * 128],
                            ident[:],
                        )
                    with nc.allow_low_precision(reason="fp8 VT"):
                        nc.vector.tensor_copy(
                            vt_sb[d][:, 4 * g:4 * (g + 1), :], ps[:]
                        )
                else:
                    for jj in range(4):
                        kc = 4 * g + jj
                        nc.sync.dma_start_transpose(
                            vtb[d][:, kc, :],
                            vtmps[d][:, kc * 128:(kc + 1) * 128],
                        )
                    with nc.allow_low_precision(reason="fp8 VT"):
                        nc.gpsimd.tensor_copy(
                            vt_sb[d][:, 4 * g:4 * (g + 1), :],
                            vtb[d][:, 4 * g:4 * (g + 1), :],
                        )

            # ---- attention ----
            def emit_S(d, qt, g):
                """S^T matmuls for one k-chunk group -> st psum tile."""
                qs = q_sb[d][:, qt * QW:(qt + 1) * QW]
                st = pst.tile([128, gj, QW], f32, tag="st", name="st")
                for jj in range(gj):
                    j = gj * g + jj
                    nc.tensor.matmul(
                        st[:, jj, :],
                        k_sb[d][:, j * 128:(j + 1) * 128],
                        qs,
                        start=True, stop=True,
                    )
                return st

            def emit_body(d, qt, st_pre, nxt):
                """exp + O/dn accumulation for one (qt, d). st_pre holds the
                2 pre-emitted S tiles (groups 0/1); the loop keeps 2 groups
                of S prefetched, crossing into segment `nxt` at the end so
                the exp engines never drain at segment boundaries.
                Returns (ot, dn, next segment's prefetched S tiles)."""
                ot = pot.tile([128, QW], f32, tag="ot", name="ot")
                dn = pshared.tile([16, QW], f32, tag="sh", name="dn")
                sts = list(st_pre)
                nxt_pre = []
                dn_pending = []   # (ex tile, group, flush-at group)
                ex_hold = []

                def emit_O(g, ext):
                    nc.tensor.matmul(
                        ot[:],
                        vt_sb[d][:, gj * g:gj * (g + 1), :],
                        ext[:],
                        start=(g == 0), stop=(g == ngrp - 1),
                        perf_mode=DR,
                    )

                def emit_dn(ext, gd):
                    nc.tensor.matmul(
                        dn[:], ones2[:], ext[:],
                        start=(gd == 0), stop=(gd == ngrp - 1),
                        perf_mode=DR,
                    )

                for g in range(ngrp):
                    if g + 2 < ngrp:
                        sts.append(emit_S(d, qt, g + 2))
                    elif nxt is not None:
                        nq, nd = nxt
                        nxt_pre.append(emit_S(nd, nq, g + 2 - ngrp))
                    # dn matmuls trail so the in-order PE never waits on exp
                    while dn_pending and dn_pending[0][2] <= g:
                        ext, gd, _ = dn_pending.pop(0)
                        emit_dn(ext, gd)
                    st_cur = sts[g]
                    ex = sex.tile([128, gj, QW], fp8, tag="ex", name="ex")
                    if (g % ngrp) in DVE_GROUPS:
                        with nc.allow_low_precision(reason="fast exp8"):
                            nc.vector.tensor_scalar(
                                ex[:].bitcast(u8), st_cur[:],
                                FE8_A, FE8_B, ALU.mult, ALU.add,
                            )
                    else:
                        nc.scalar.activation(ex[:], st_cur[:], AFT.Exp)
                    # the first 2 O-matmuls run at group 2: the fresh ot
                    # psum bank may still be read by the previous segment's
                    # normalize mul, and this grants 2 groups of slack
                    if g < 2:
                        ex_hold.append(ex)
                    else:
                        if g == 2:
                            for gh, exh in enumerate(ex_hold):
                                emit_O(gh, exh)
                        emit_O(g, ex)
                    dn_pending.append((ex, g, g + DN_DELAY))
                for ext, gd, _ in dn_pending:
                    emit_dn(ext, gd)
                return ot, dn, nxt_pre

            def emit_tail(ot, dn):
                """softmax normalize -> osb (f32r SBUF)."""
                rc = smisc.tile([1, QW], f32, tag="rc", name="rc")
                nc.vector.reciprocal_approx_fast(rc[:], dn[0:1, :])
                rcr = smisc.tile([1, QW], f32r, tag="rcr", name="rcr")
                with nc.allow_low_precision(reason="f32r recip"):
                    nc.vector.tensor_copy(rcr[:], rc[:])
                # 1/dn broadcast via Kc=1 matmul
                bc_ps = pshared.tile([128, QW], f32, tag="sh", name="bc")
                nc.tensor.matmul(
                    bc_ps[:], ones_r[:], rcr[:],
                    start=True, stop=True,
                )
                bc_sb = sbc.tile([128, QW], f32r, tag="bcs", name="bcs")
                nc.vector.tensor_copy(bc_sb[:], bc_ps[:])
                osb = sot.tile([128, QW], f32r, tag="osb", name="osb")
                with nc.allow_low_precision(reason="f32r osb"):
                    nc.vector.tensor_mul(osb[:], ot[:], bc_sb[:])
                return osb

            def emit_final(qt, ot_sbs):
                for dch in range(2):
                    yp = pshared.tile([128, QW], f32, tag="sh", name="yp")
                    for d in (0, 1):
                        nc.tensor.matmul(
                            yp[:],
                            wp[:, d, dch * 128:(dch + 1) * 128],
                            ot_sbs[d][:],
                            start=(d == 0), stop=(d == 1),
                        )
                    ysb = smisc.tile([128, QW], f32, tag="ysb", name="ysb")
                    nc.scalar.activation(
                        ysb[:], yp[:], AFT.Relu,
                        bias=bshf[:, dch, :], scale=binv[:, dch, :],
                    )
                    nc.sync.dma_start(
                        y_d[dch * 128:(dch + 1) * 128,
                            qt * QW:(qt + 1) * QW],
                        ysb[:],
                    )

            # ---- schedule ----
            # dir-a K0/Q0 + first S-tile so exp starts as soon as the first
            # feature pieces land; remaining dir-a projections follow; dir-b
            # projection tiles slip between the first dir-a segments in
            # small chunks so PE tail-stall slack absorbs them.
            emit_proj(0, "k", 0)
            emit_proj(0, "q", 0)
            st_next0 = emit_S(0, 0, 0)
            emit_proj(0, "v", 0)
            emit_trans(0, 0)
            for nt in range(1, nqt):
                emit_proj(0, "k", nt)
                emit_proj(0, "v", nt)
                emit_proj(0, "q", nt)
            for nt in range(1, nqt):
                emit_trans(0, nt)

            def fill_k1():
                for nt in range(nqt):
                    emit_proj(1, "k", nt)

            def fill_v1():
                for nt in range(nqt):
                    emit_proj(1, "v", nt)
                for nt in range(nqt):
                    emit_trans(1, nt)

            def fill_q1():
                for nt in range(nqt):
                    emit_proj(1, "q", nt)

            fillers = [fill_k1, fill_v1, fill_q1]
            segs = [(qt, 0) for qt in range(nqt)] + [(qt, 1) for qt in range(nqt)]

            st_pre = [st_next0, emit_S(0, 0, 1)]
            pending = {}          # qt -> {d: osb}
            for i, (qt, d) in enumerate(segs):
                nxt = segs[i + 1] if i + 1 < len(segs) else None
                ot, dn, st_pre = emit_body(d, qt, st_pre, nxt)
                osb = emit_tail(ot, dn)
                pending.setdefault(qt, {})[d] = osb
                # finals run one segment late so the yp matmuls always have
                # a full segment of PE work as cover for the tail chain
                if d == 1 and qt > 0:
                    emit_final(qt - 1, pending.pop(qt - 1))
                if i < len(fillers):
                    fillers[i]()
            emit_final(nqt - 1, pending.pop(nqt - 1))
    nc.compile()
    return nc


def _to_bf16_bits(x):
    u = np.ascontiguousarray(x, np.float32).view(np.uint32)
    r = ((u + 0x7FFF + ((u >> 16) & 1)) >> 16).astype(np.uint16)
    return r


def _to_fp8e4_bits(x):
    import ml_dtypes

    return np.ascontiguousarray(x, np.float32).astype(
        ml_dtypes.float8_e4m3fn).view(np.uint8)


def _host_prep(inputs, n=N):
    f_rgb = _to_bf16_bits(inputs["f_rgb"].reshape(B, C, n))
    f_pl = _to_bf16_bits(inputs["f_pl"].reshape(B, C, n))

    def T(w, scale=1.0):
        return np.ascontiguousarray(scale * np.asarray(w, np.float32).T)

    def T16(w, scale=1.0):
        return _to_bf16_bits(T(w, scale))

    wp = np.asarray(inputs["w_proj"], np.float32)
    inv = np.asarray(inputs["bn_gamma"], np.float32) / np.sqrt(
        np.asarray(inputs["bn_var"], np.float32) + 1e-5)
    shift = (np.asarray(inputs["bn_beta"], np.float32)
             - np.asarray(inputs["bn_mean"], np.float32) * inv
             + inv * (wp[:, :E] @ np.asarray(inputs["b_v_pl"], np.float32)
                      + wp[:, E:] @ np.asarray(inputs["b_v_rgb"], np.float32)))

    shared = {
        "wq_a": T16(inputs["w_q_rgb"], SCALE),
        "wk_a": T16(inputs["w_k_pl"]),
        "wv_a": T16(inputs["w_v_pl"]),
        "wq_b": T16(inputs["w_q_pl"], SCALE),
        "wk_b": T16(inputs["w_k_rgb"]),
        "wv_b": T16(inputs["w_v_rgb"]),
        "wp": T(wp),
        "bq_a": (SCALE * np.asarray(inputs["b_q_rgb"], np.float32))
        .reshape(E, 1).copy(),
        "bk_a": np.asarray(inputs["b_k_pl"], np.float32).reshape(E, 1).copy(),
        "bq_b": (SCALE * np.asarray(inputs["b_q_pl"], np.float32))
        .reshape(E, 1).copy(),
        "bk_b": np.asarray(inputs["b_k_rgb"], np.float32).reshape(E, 1).copy(),
        "bn_inv": inv.reshape(OUT, 1).copy(),
        "bn_shf": shift.reshape(OUT, 1).copy(),
        "ones2": _to_fp8e4_bits(np.ones((E, 2, 16), np.float32)),
        "ones_r": np.ones((1, E), np.float32),
        "ident": _to_bf16_bits(np.eye(E, dtype=np.float32)),
    }
    in_maps = []
    for b in range(B):
        m = dict(shared)
        m["f_a"] = f_rgb[b]
        m["f_b"] = f_pl[b]
        in_maps.append(m)
    return in_maps


def kernel(**inputs):
    from concourse import bass_utils

    if "nc" not in _CACHE:
        _CACHE["nc"] = build_nc()
    nc = _CACHE["nc"]
    in_maps = _host_prep(inputs)
    res = bass_utils.run_bass_kernel_spmd(nc, in_maps, core_ids=list(range(B)))
    out = np.stack([res.results[b]["y"] for b in range(B)], axis=0)
    return out.reshape(B, OUT, H, W).astype(np.float32)


if __name__ == "__main__":
    pass


# revision 8
# speedup vs baseline: 1.1658x; 1.0009x over previous
"""CrossModalAttention TRN2 kernel (v2: fp8 DoubleRow attention).

Strategy (data-parallel over batch, one batch element per NeuronCore):
  dir a: q from rgb, k/v from pl;  dir b: q from pl, k/v from rgb.
  Per direction:
    Q  = scale*(Wq @ f_q + bq)        [128 e, N] bf16 (scale folded into W,b)
    K  = Wk @ f_k + bk                [128 e, N] bf16
    VT = (Wv @ f_k)^T                 [N k, 128 e] fp8e4m3 (v-bias folded
                                      into the BN shift host-side)
    per q-tile (512 wide), per group g of 2 k-chunks:
      S^T_g = K_g^T @ Q_tile          [128 k, 2, 512 q]  (PSUM f32)
      E_g   = exp(S^T_g) -> fp8       ScalarE for most groups; VectorE
                                      computes e4m3 bits directly via the
                                      round(x*8*log2e + 55.5) affine trick
                                      for DVE_GROUPS (engine balance)
      OT   += VT_g^T @ E_g            one fp8 DoubleRow matmul (256-row
                                      contraction, 2x col rate)
      dn   += ones^T @ E_g            one fp8 DoubleRow matmul (weight padded
                                      to 16 cols for the lw step%16 rule),
                                      delayed 3 groups to stay off the
                                      critical path
      OT_norm = OT * bcast(1/dn)      reciprocal_approx_fast on DVE; bcast
                                      via Kc=1 rank-1 matmul
  y = Wp_a @ OT_a + Wp_b @ OT_b ; out = relu(inv*y + shift)  (BN folded)

Schedule: dir-a K/V features DMA first; dir-a projections then dir-a
attention start immediately, with dir-b projections slipped between the
first dir-a segments so the exp engines start ~40us earlier than a
proj-everything-first order.
"""

import sys

sys.path.insert(0, "/opt/trn_rl_repo")

import numpy as np

B = 8
C = 256
E = 128
OUT = 256
H = W = 64
N = H * W
QW = 512
SCALE = float(E) ** -0.5

LOG2E = 1.4426950408889634
FE8_A = 8.0 * LOG2E          # e4m3 bits = round(s*FE8_A + FE8_B)
FE8_B = 7.0 * 8.0 - 0.5      # HW float->uint8 rounds to nearest; c=-0.5
# groups (of 16 per segment) whose exp runs on DVE instead of ScalarE
DVE_GROUPS = frozenset({4, 8, 12, 15})

_CACHE = {}


def _patch_tail_drain(tile_mod, mybir):
    # This walrus build encodes Drain as CTRL_NO_STRUCT with a single
    # sync-wait slot; split the TileContext tail drain's waits across
    # one drain instruction per semaphore.
    if getattr(tile_mod.TileContext, "_drain_patched", False):
        return
    from concourse.vector_clock import ScopedClock

    def _drain_and_barrier(self, tick_clock, wait_clock):
        nc = self.nc
        drain_inst = nc.sync.drain()
        wait_clock.add_sem_waits(
            drain_inst.ins, ScopedClock({None: tick_clock.global_clock})
        )
        si = drain_inst.ins.sync_info
        if si is not None and si.on_wait and len(si.on_wait) > 1:
            waits = list(si.on_wait)
            drain_inst.ins.sync_info = mybir.SyncInfo(
                on_wait=[waits[0]], on_update=list(si.on_update or [])
            )
            for w in waits[1:]:
                d2 = nc.sync.drain()
                d2.ins.sync_info = mybir.SyncInfo(on_wait=[w], on_update=[])
        nc.all_engine_barrier()
        popped = nc._tile_sem_poison_stack.pop()
        assert popped is self._sem_poison
        nc.clear_and_free_semaphores(list(self.sems.allocated().values()))
        nc.all_engine_barrier()

    tile_mod.TileContext._drain_and_barrier = _drain_and_barrier
    tile_mod.TileContext._drain_patched = True


def build_nc(n=N, debug=False):
    """Build the single-core Bass program. n = spatial size (4096 full)."""
    import concourse.bacc as bacc
    import concourse.tile as tile
    from concourse import mybir

    f32 = mybir.dt.float32
    f32r = mybir.dt.float32r
    bf16 = mybir.dt.bfloat16
    fp8 = mybir.dt.float8e4
    u8 = mybir.dt.uint8
    AFT = mybir.ActivationFunctionType
    ALU = mybir.AluOpType
    DR = mybir.MatmulPerfMode.DoubleRow

    gj = 2                  # k-chunks per PSUM S-tile / exp instruction
    O_DELAY = 4             # groups the O matmuls trail by (covers prev tail)
    nqt = n // QW
    nkc = n // 128
    ngrp = nkc // gj        # exp groups per segment
    DN_DELAY = 2            # dn trails so prev-seg bcMM can slot in first

    nc = bacc.Bacc(trn_type="TRN2", target_bir_lowering=False, debug=False)

    def din(name, shape, dt_=f32):
        return nc.dram_tensor(name, shape, dt_, kind="ExternalInput").ap()

    u16 = mybir.dt.uint16
    f_a_d = din("f_a", [C, n], u16)   # rgb features bf16 bits (q-side of a)
    f_b_d = din("f_b", [C, n], u16)   # pl features bf16 bits
    wq_a_d = din("wq_a", [C, E], u16)  # scale * W_q_rgb^T (bf16 bits)
    wk_a_d = din("wk_a", [C, E], u16)  # W_k_pl^T
    wv_a_d = din("wv_a", [C, E], u16)  # W_v_pl^T
    wq_b_d = din("wq_b", [C, E], u16)  # scale * W_q_pl^T
    wk_b_d = din("wk_b", [C, E], u16)  # W_k_rgb^T
    wv_b_d = din("wv_b", [C, E], u16)  # W_v_rgb^T
    wp_d = din("wp", [2 * E, OUT])    # w_proj^T
    bq_a_d = din("bq_a", [E, 1])      # scale * b_q_rgb
    bk_a_d = din("bk_a", [E, 1])      # b_k_pl
    bq_b_d = din("bq_b", [E, 1])      # scale * b_q_pl
    bk_b_d = din("bk_b", [E, 1])      # b_k_rgb
    inv_d = din("bn_inv", [OUT, 1])
    shf_d = din("bn_shf", [OUT, 1])
    ones2_d = din("ones2", [E, 2, 16], mybir.dt.uint8)  # fp8 ones, padded
    ones_r_d = din("ones_r", [1, E])
    ident_d = din("ident", [E, E], mybir.dt.uint16)     # bf16 bits
    y_d = nc.dram_tensor("y", [OUT, n], f32, kind="ExternalOutput").ap()

    with tile.TileContext(nc) as tc:
        # PSUM: st 3x2 banks + ot 1 + dn/bc/yp shared 1 = 8 banks.
        with tc.tile_pool(name="const", bufs=1) as const, \
             tc.tile_pool(name="qkv", bufs=1) as qkv, \
             tc.tile_pool(name="feat", bufs=1) as feat, \
             tc.tile_pool(name="pst", bufs=3, space="PSUM") as pst, \
             tc.tile_pool(name="pot", bufs=1, space="PSUM") as pot, \
             tc.tile_pool(name="pshared", bufs=1, space="PSUM") as pshared, \
             tc.tile_pool(name="sex", bufs=8) as sex, \
             tc.tile_pool(name="sot", bufs=10) as sot, \
             tc.tile_pool(name="sbc", bufs=3) as sbc, \
             tc.tile_pool(name="smisc", bufs=4) as smisc:

            # ---- DMA: dir-a critical path first ----
            def wload(d, nm):
                t = const.tile([128, 2, E], bf16, name=nm, tag=nm)
                nc.sync.dma_start(t[:], d.rearrange("(c p) e -> p c e", p=128).bitcast(bf16))
                return t

            def vload(d, shape, nm, dt_=None, eng=None):
                t = const.tile(shape, dt_ or f32, name=nm, tag=nm)
                (eng or nc.sync).dma_start(t[:], d.bitcast(dt_) if dt_ else d)
                return t

            # dir-a critical path: wk_a (K0), wq_a (Q0) before the features
            wk = {0: wload(wk_a_d, "wka")}
            wq = {0: wload(wq_a_d, "wqa")}
            bk = {0: vload(bk_a_d, [E, 1], "bka")}
            bq = {0: vload(bq_a_d, [E, 1], "bqa")}

            fsb = {
                name: feat.tile([128, 2, n], bf16, tag=f"f{name}",
                                name=f"f_{name}")
                for name in ("a", "b")
            }
            npc = max(1, n // 512)    # 512-col pieces
            # First pieces of BOTH tensors first (K0 needs f_b[0], Q0 needs
            # f_a[0]); rest streams behind on the two HWDGE queues (sync +
            # scalar). gpsimd stays DMA-free so its tail drain is cheap.
            def fpiece(name, pc):
                d_src = f_b_d if name == "b" else f_a_d
                lo, hi = pc * (n // npc), (pc + 1) * (n // npc)
                for cc in range(2):
                    eng = nc.sync if cc == 0 else nc.scalar
                    eng.dma_start(
                        fsb[name][:, cc, lo:hi],
                        d_src[cc * 128:(cc + 1) * 128, lo:hi].bitcast(bf16),
                    )

            fpiece("b", 0)
            fpiece("a", 0)
            fpiece("b", 1)
            fpiece("a", 1)
            wv = {0: wload(wv_a_d, "wva")}
            ident = vload(ident_d, [E, E], "idt", bf16)
            ones2 = vload(ones2_d, [E, 2, 16], "on2", fp8)
            ones_r = vload(ones_r_d, [1, E], "onr", f32r)
            for pc in range(2, npc):
                fpiece("b", pc)
                fpiece("a", pc)
            # dir-b weights + late consts
            wk[1] = wload(wk_b_d, "wkb")
            wv[1] = wload(wv_b_d, "wvb")
            wq[1] = wload(wq_b_d, "wqb")
            bk[1] = vload(bk_b_d, [E, 1], "bkb")
            bq[1] = vload(bq_b_d, [E, 1], "bqb")
            wp = const.tile([128, 2, OUT], f32r, name="wp", tag="wp")
            nc.sync.dma_start(wp[:], wp_d.rearrange("(c p) e -> p c e", p=128).bitcast(f32r))
            binv = const.tile([128, 2, 1], f32, name="binv", tag="binv")
            nc.sync.dma_start(binv[:], inv_d.rearrange("(c p) e -> p c e", p=128))
            bshf = const.tile([128, 2, 1], f32, name="bshf", tag="bshf")
            nc.sync.dma_start(bshf[:], shf_d.rearrange("(c p) e -> p c e", p=128))

            # ---- per-direction activations ----
            q_sb = {d: qkv.tile([128, n], bf16, tag=f"q{d}", name=f"q_sb{d}") for d in (0, 1)}
            k_sb = {d: qkv.tile([128, n], bf16, tag=f"k{d}", name=f"k_sb{d}") for d in (0, 1)}
            vt_sb = {d: qkv.tile([128, nkc, 128], fp8, tag=f"v{d}", name=f"vt_sb{d}")
                     for d in (0, 1)}
            vtb = {d: qkv.tile([128, nkc, 128], bf16, tag=f"vb{d}", name=f"vtb{d}")
                   for d in (0, 1)}
            vtmps = {d: feat.tile([128, n], bf16, tag=f"vtmp{d}", name=f"vtmp{d}")
                     for d in (0, 1)}

            def emit_proj(d, kind, nt):
                """One 512-wide projection tile: kind in k/v/q."""
                fq = fsb["a"] if d == 0 else fsb["b"]
                fk = fsb["b"] if d == 0 else fsb["a"]
                wt, bias, dst, src = {
                    "k": (wk[d], bk[d], k_sb[d], fk),
                    "v": (wv[d], None, vtmps[d], fk),
                    "q": (wq[d], bq[d], q_sb[d], fq),
                }[kind]
                ps = pst.tile([128, QW], f32, tag="st", name="psp")
                for cc in range(2):
                    nc.tensor.matmul(
                        ps[:],
                        wt[:, cc, :],
                        src[:, cc, nt * QW:(nt + 1) * QW],
                        start=(cc == 0),
                        stop=(cc == 1),
                    )
                with nc.allow_low_precision(reason="bf16 proj"):
                    if bias is None:
                        nc.vector.tensor_copy(
                            dst[:, nt * QW:(nt + 1) * QW], ps[:])
                    else:
                        nc.vector.tensor_scalar_add(
                            dst[:, nt * QW:(nt + 1) * QW], ps[:], bias[:])

            def emit_trans(d, g):
                """Transpose 4 v chunks -> vt fp8. dir a: PE transpose (low
                latency, feeds the very first segments). dir b: DMA xbar on
                the idle sync queue + gpsimd cast (off the PE; its segments
                start >100us later). Never touch the scalar queue: DMA
                dispatch there stalls the exp stream."""
                if d == 0:
                    ps = pst.tile([128, QW], bf16, tag="st", name="psvt")
                    for jj in range(4):
                        kc = 4 * g + jj
                        nc.tensor.transpose(
                            ps[:, jj * 128:(jj + 1) * 128],
                            vtmps[d][:, kc * 128:(kc + 1) * 128],
                            ident[:],
                        )
                    with nc.allow_low_precision(reason="fp8 VT"):
                        nc.vector.tensor_copy(
                            vt_sb[d][:, 4 * g:4 * (g + 1), :], ps[:]
                        )
                else:
                    for jj in range(4):
                        kc = 4 * g + jj
                        nc.sync.dma_start_transpose(
                            vtb[d][:, kc, :],
                            vtmps[d][:, kc * 128:(kc + 1) * 128],
                        )
                    with nc.allow_low_precision(reason="fp8 VT"):
                        nc.gpsimd.tensor_copy(
                            vt_sb[d][:, 4 * g:4 * (g + 1), :],
                            vtb[d][:, 4 * g:4 * (g + 1), :],
                        )

            # ---- attention ----
            def emit_S(d, qt, g):
                """S^T matmuls for one k-chunk group -> st psum tile."""
                qs = q_sb[d][:, qt * QW:(qt + 1) * QW]
                st = pst.tile([128, gj, QW], f32, tag="st", name="st")
                for jj in range(gj):
                    j = gj * g + jj
                    nc.tensor.matmul(
                        st[:, jj, :],
                        k_sb[d][:, j * 128:(j + 1) * 128],
                        qs,
                        start=True, stop=True,
                    )
                return st

            def emit_body(d, qt, st_pre, nxt, deferred=None):
                """exp + O/dn accumulation for one (qt, d). st_pre holds the
                2 pre-emitted S tiles (groups 0/1); the loop keeps 2 groups
                of S prefetched, crossing into segment `nxt` at the end so
                the exp engines never drain at segment boundaries.
                Returns (ot, dn, next segment's prefetched S tiles)."""
                ot = pot.tile([128, QW], f32, tag="ot", name="ot")
                dn = pshared.tile([16, QW], f32, tag="sh", name="dn")
                sts = list(st_pre)
                nxt_pre = []
                dn_pending = []   # (ex tile, group, flush-at group)
                ex_hold = []

                def emit_O(g, ext):
                    nc.tensor.matmul(
                        ot[:],
                        vt_sb[d][:, gj * g:gj * (g + 1), :],
                        ext[:],
                        start=(g == 0), stop=(g == ngrp - 1),
                        perf_mode=DR,
                    )

                def emit_dn(ext, gd):
                    nc.tensor.matmul(
                        dn[:], ones2[:], ext[:],
                        start=(gd == 0), stop=(gd == ngrp - 1),
                        perf_mode=DR,
                    )

                for g in range(ngrp):
                    if g == 2 and deferred is not None:
                        deferred()   # prev segment's bcMM/bc_copy/osb
                    if g + 2 < ngrp:
                        sts.append(emit_S(d, qt, g + 2))
                    elif nxt is not None:
                        nq, nd = nxt
                        nxt_pre.append(emit_S(nd, nq, g + 2 - ngrp))
                    # dn matmuls trail so the in-order PE never waits on exp
                    while dn_pending and dn_pending[0][2] <= g:
                        ext, gd, _ = dn_pending.pop(0)
                        emit_dn(ext, gd)
                    st_cur = sts[g]
                    ex = sex.tile([128, gj, QW], fp8, tag="ex", name="ex")
                    if (g % ngrp) in DVE_GROUPS:
                        with nc.allow_low_precision(reason="fast exp8"):
                            nc.vector.tensor_scalar(
                                ex[:].bitcast(u8), st_cur[:],
                                FE8_A, FE8_B, ALU.mult, ALU.add,
                            )
                    else:
                        nc.scalar.activation(ex[:], st_cur[:], AFT.Exp)
                    # the first O_DELAY O-matmuls are held: the fresh ot
                    # psum bank is read by the previous segment's deferred
                    # normalize mul (emitted at group 2), and this grants
                    # the slack for that handoff
                    if g < O_DELAY:
                        ex_hold.append(ex)
                    else:
                        if g == O_DELAY:
                            for gh, exh in enumerate(ex_hold):
                                emit_O(gh, exh)
                        emit_O(g, ex)
                    dn_pending.append((ex, g, g + DN_DELAY))
                for ext, gd, _ in dn_pending:
                    emit_dn(ext, gd)
                return ot, dn, nxt_pre

            def emit_tail_head(dn):
                """reciprocal of dn, right at segment end (DVE only)."""
                rc = smisc.tile([1, QW], f32, tag="rc", name="rc")
                nc.vector.reciprocal_approx_fast(rc[:], dn[0:1, :])
                rcr = smisc.tile([1, QW], f32r, tag="rcr", name="rcr")
                with nc.allow_low_precision(reason="f32r recip"):
                    nc.vector.tensor_copy(rcr[:], rc[:])
                # create the bc psum tile now so the pshared rotation order
                # stays dn(i) -> bc(i) -> dn(i+1); its matmul is deferred
                bc_ps = pshared.tile([128, QW], f32, tag="sh", name="bc")
                return rcr, bc_ps

            def emit_tail_rest(ot, rcr, bc_ps):
                """broadcast + normalize, emitted 2 groups into the next
                segment so the PE never waits on the reciprocal chain."""
                nc.tensor.matmul(
                    bc_ps[:], ones_r[:], rcr[:],
                    start=True, stop=True,
                )
                bc_sb = sbc.tile([128, QW], f32r, tag="bcs", name="bcs")
                nc.vector.tensor_copy(bc_sb[:], bc_ps[:])
                osb = sot.tile([128, QW], f32r, tag="osb", name="osb")
                with nc.allow_low_precision(reason="f32r osb"):
                    nc.vector.tensor_mul(osb[:], ot[:], bc_sb[:])
                return osb

            def emit_final(qt, ot_sbs):
                for dch in range(2):
                    yp = pshared.tile([128, QW], f32, tag="sh", name="yp")
                    for d in (0, 1):
                        nc.tensor.matmul(
                            yp[:],
                            wp[:, d, dch * 128:(dch + 1) * 128],
                            ot_sbs[d][:],
                            start=(d == 0), stop=(d == 1),
                        )
                    ysb = smisc.tile([128, QW], f32, tag="ysb", name="ysb")
                    nc.scalar.activation(
                        ysb[:], yp[:], AFT.Relu,
                        bias=bshf[:, dch, :], scale=binv[:, dch, :],
                    )
                    nc.sync.dma_start(
                        y_d[dch * 128:(dch + 1) * 128,
                            qt * QW:(qt + 1) * QW],
                        ysb[:],
                    )

            # ---- schedule ----
            # dir-a K0/Q0 + first S-tile so exp starts as soon as the first
            # feature pieces land; remaining dir-a projections follow; dir-b
            # projection tiles slip between the first dir-a segments in
            # small chunks so PE tail-stall slack absorbs them.
            emit_proj(0, "k", 0)
            emit_proj(0, "q", 0)
            st_next0 = emit_S(0, 0, 0)
            emit_proj(0, "v", 0)
            emit_trans(0, 0)
            for nt in range(1, nqt):
                emit_proj(0, "k", nt)
                emit_proj(0, "v", nt)
                emit_proj(0, "q", nt)
            for nt in range(1, nqt):
                emit_trans(0, nt)

            def fill_k1():
                for nt in range(nqt):
                    emit_proj(1, "k", nt)

            def fill_v1():
                for nt in range(nqt):
                    emit_proj(1, "v", nt)
                for nt in range(nqt):
                    emit_trans(1, nt)

            def fill_q1():
                for nt in range(nqt):
                    emit_proj(1, "q", nt)

            fillers = [fill_k1, fill_v1, fill_q1]
            segs = [(qt, 0) for qt in range(nqt)] + [(qt, 1) for qt in range(nqt)]

            st_pre = [st_next0, emit_S(0, 0, 1)]
            pending = {}          # qt -> {d: osb}
            deferred = None
            for i, (qt, d) in enumerate(segs):
                nxt = segs[i + 1] if i + 1 < len(segs) else None
                ot, dn, st_pre = emit_body(d, qt, st_pre, nxt,
                                           deferred=deferred)
                # finals run one segment late; they must be emitted BEFORE
                # emit_tail_head so the pshared rotation order is
                # dn(i) -> yp -> bc(i) -> dn(i+1) (bc's matmul is deferred
                # into the next segment; yp after bc would deadlock the PE)
                if d == 1 and qt > 0:
                    emit_final(qt - 1, pending.pop(qt - 1))
                rcr, bc_ps = emit_tail_head(dn)

                def deferred(ot=ot, rcr=rcr, bc_ps=bc_ps, qt=qt, d=d):
                    pending.setdefault(qt, {})[d] = emit_tail_rest(
                        ot, rcr, bc_ps)

                if i < len(fillers):
                    fillers[i]()
            deferred()
            emit_final(nqt - 1, pending.pop(nqt - 1))
    nc.compile()
    return nc


def _to_bf16_bits(x):
    u = np.ascontiguousarray(x, np.float32).view(np.uint32)
    r = ((u + 0x7FFF + ((u >> 16) & 1)) >> 16).astype(np.uint16)
    return r


def _to_fp8e4_bits(x):
    import ml_dtypes

    return np.ascontiguousarray(x, np.float32).astype(
        ml_dtypes.float8_e4m3fn).view(np.uint8)


def _host_prep(inputs, n=N):
    f_rgb = _to_bf16_bits(inputs["f_rgb"].reshape(B, C, n))
    f_pl = _to_bf16_bits(inputs["f_pl"].reshape(B, C, n))

    def T(w, scale=1.0):
        return np.ascontiguousarray(scale * np.asarray(w, np.float32).T)

    def T16(w, scale=1.0):
        return _to_bf16_bits(T(w, scale))

    wp = np.asarray(inputs["w_proj"], np.float32)
    inv = np.asarray(inputs["bn_gamma"], np.float32) / np.sqrt(
        np.asarray(inputs["bn_var"], np.float32) + 1e-5)
    shift = (np.asarray(inputs["bn_beta"], np.float32)
             - np.asarray(inputs["bn_mean"], np.float32) * inv
             + inv * (wp[:, :E] @ np.asarray(inputs["b_v_pl"], np.float32)
                      + wp[:, E:] @ np.asarray(inputs["b_v_rgb"], np.float32)))

    shared = {
        "wq_a": T16(inputs["w_q_rgb"], SCALE),
        "wk_a": T16(inputs["w_k_pl"]),
        "wv_a": T16(inputs["w_v_pl"]),
        "wq_b": T16(inputs["w_q_pl"], SCALE),
        "wk_b": T16(inputs["w_k_rgb"]),
        "wv_b": T16(inputs["w_v_rgb"]),
        "wp": T(wp),
        "bq_a": (SCALE * np.asarray(inputs["b_q_rgb"], np.float32))
        .reshape(E, 1).copy(),
        "bk_a": np.asarray(inputs["b_k_pl"], np.float32).reshape(E, 1).copy(),
        "bq_b": (SCALE * np.asarray(inputs["b_q_pl"], np.float32))
        .reshape(E, 1).copy(),
        "bk_b": np.asarray(inputs["b_k_rgb"], np.float32).reshape(E, 1).copy(),
        "bn_inv": inv.reshape(OUT, 1).copy(),
        "bn_shf": shift.reshape(OUT, 1).copy(),
        "ones2": _to_fp8e4_bits(np.ones((E, 2, 16), np.float32)),
        "ones_r": np.ones((1, E), np.float32),
        "ident": _to_bf16_bits(np.eye(E, dtype=np.float32)),
    }
    in_maps = []
    for b in range(B):
        m = dict(shared)
        m["f_a"] = f_rgb[b]
        m["f_b"] = f_pl[b]
        in_maps.append(m)
    return in_maps


def kernel(**inputs):
    from concourse import bass_utils

    if "nc" not in _CACHE:
        _CACHE["nc"] = build_nc()
    nc = _CACHE["nc"]
    in_maps = _host_prep(inputs)
    res = bass_utils.run_bass_kernel_spmd(nc, in_maps, core_ids=list(range(B)))
    out = np.stack([res.results[b]["y"] for b in range(B)], axis=0)
    return out.reshape(B, OUT, H, W).astype(np.float32)


if __name__ == "__main__":
    pass


# revision 10
# speedup vs baseline: 1.1680x; 1.0018x over previous
"""CrossModalAttention TRN2 kernel (v2: fp8 DoubleRow attention).

Strategy (data-parallel over batch, one batch element per NeuronCore):
  dir a: q from rgb, k/v from pl;  dir b: q from pl, k/v from rgb.
  Per direction:
    Q  = scale*(Wq @ f_q + bq)        [128 e, N] bf16 (scale folded into W,b)
    K  = Wk @ f_k + bk                [128 e, N] bf16
    VT = (Wv @ f_k)^T                 [N k, 128 e] fp8e4m3 (v-bias folded
                                      into the BN shift host-side)
    per q-tile (512 wide), per group g of 2 k-chunks:
      S^T_g = K_g^T @ Q_tile          [128 k, 2, 512 q]  (PSUM f32)
      E_g   = exp(S^T_g) -> fp8       ScalarE for most groups; VectorE
                                      computes e4m3 bits directly via the
                                      round(x*8*log2e + 55.5) affine trick
                                      for DVE_GROUPS (engine balance)
      OT   += VT_g^T @ E_g            one fp8 DoubleRow matmul (256-row
                                      contraction, 2x col rate)
      dn   += ones^T @ E_g            one fp8 DoubleRow matmul (weight padded
                                      to 16 cols for the lw step%16 rule),
                                      delayed 3 groups to stay off the
                                      critical path
      OT_norm = OT * bcast(1/dn)      reciprocal_approx_fast on DVE; bcast
                                      via Kc=1 rank-1 matmul
  y = Wp_a @ OT_a + Wp_b @ OT_b ; out = relu(inv*y + shift)  (BN folded)

Schedule: dir-a K/V features DMA first; dir-a projections then dir-a
attention start immediately, with dir-b projections slipped between the
first dir-a segments so the exp engines start ~40us earlier than a
proj-everything-first order.
"""

import sys

sys.path.insert(0, "/opt/trn_rl_repo")

import numpy as np

B = 8
C = 256
E = 128
OUT = 256
H = W = 64
N = H * W
QW = 512
SCALE = float(E) ** -0.5

LOG2E = 1.4426950408889634
FE8_A = 8.0 * LOG2E          # e4m3 bits = round(s*FE8_A + FE8_B)
FE8_B = 7.0 * 8.0 - 0.5      # HW float->uint8 rounds to nearest; c=-0.5
# groups (of 16 per segment) whose exp runs on DVE instead of ScalarE
DVE_GROUPS = frozenset({2, 5, 8, 11, 13, 15})

_CACHE = {}


def _patch_tail_drain(tile_mod, mybir):
    # This walrus build encodes Drain as CTRL_NO_STRUCT with a single
    # sync-wait slot; split the TileContext tail drain's waits across
    # one drain instruction per semaphore.
    if getattr(tile_mod.TileContext, "_drain_patched", False):
        return
    from concourse.vector_clock import ScopedClock

    def _drain_and_barrier(self, tick_clock, wait_clock):
        nc = self.nc
        drain_inst = nc.sync.drain()
        wait_clock.add_sem_waits(
            drain_inst.ins, ScopedClock({None: tick_clock.global_clock})
        )
        si = drain_inst.ins.sync_info
        if si is not None and si.on_wait and len(si.on_wait) > 1:
            waits = list(si.on_wait)
            drain_inst.ins.sync_info = mybir.SyncInfo(
                on_wait=[waits[0]], on_update=list(si.on_update or [])
            )
            for w in waits[1:]:
                d2 = nc.sync.drain()
                d2.ins.sync_info = mybir.SyncInfo(on_wait=[w], on_update=[])
        nc.all_engine_barrier()
        popped = nc._tile_sem_poison_stack.pop()
        assert popped is self._sem_poison
        nc.clear_and_free_semaphores(list(self.sems.allocated().values()))
        nc.all_engine_barrier()

    tile_mod.TileContext._drain_and_barrier = _drain_and_barrier
    tile_mod.TileContext._drain_patched = True


def build_nc(n=N, debug=False):
    """Build the single-core Bass program. n = spatial size (4096 full)."""
    import concourse.bacc as bacc
    import concourse.tile as tile
    from concourse import mybir

    f32 = mybir.dt.float32
    f32r = mybir.dt.float32r
    bf16 = mybir.dt.bfloat16
    fp8 = mybir.dt.float8e4
    u8 = mybir.dt.uint8
    AFT = mybir.ActivationFunctionType
    ALU = mybir.AluOpType
    DR = mybir.MatmulPerfMode.DoubleRow

    gj = 2                  # k-chunks per PSUM S-tile / exp instruction
    O_DELAY = 4             # groups the O matmuls trail by (covers prev tail)
    nqt = n // QW
    nkc = n // 128
    ngrp = nkc // gj        # exp groups per segment
    DN_DELAY = 2            # dn trails so prev-seg bcMM can slot in first

    nc = bacc.Bacc(trn_type="TRN2", target_bir_lowering=False, debug=False)

    def din(name, shape, dt_=f32):
        return nc.dram_tensor(name, shape, dt_, kind="ExternalInput").ap()

    u16 = mybir.dt.uint16
    f_a_d = din("f_a", [C, n], u16)   # rgb features bf16 bits (q-side of a)
    f_b_d = din("f_b", [C, n], u16)   # pl features bf16 bits
    wq_a_d = din("wq_a", [C, E], u16)  # scale * W_q_rgb^T (bf16 bits)
    wk_a_d = din("wk_a", [C, E], u16)  # W_k_pl^T
    wv_a_d = din("wv_a", [C, E], u16)  # W_v_pl^T
    wq_b_d = din("wq_b", [C, E], u16)  # scale * W_q_pl^T
    wk_b_d = din("wk_b", [C, E], u16)  # W_k_rgb^T
    wv_b_d = din("wv_b", [C, E], u16)  # W_v_rgb^T
    wp_d = din("wp", [2 * E, OUT])    # w_proj^T
    bq_a_d = din("bq_a", [E, 1])      # scale * b_q_rgb
    bk_a_d = din("bk_a", [E, 1])      # b_k_pl
    bq_b_d = din("bq_b", [E, 1])      # scale * b_q_pl
    bk_b_d = din("bk_b", [E, 1])      # b_k_rgb
    inv_d = din("bn_inv", [OUT, 1])
    shf_d = din("bn_shf", [OUT, 1])
    ones2_d = din("ones2", [E, 2, 16], mybir.dt.uint8)  # fp8 ones, padded
    ones_r_d = din("ones_r", [1, E])
    ident_d = din("ident", [E, E], mybir.dt.uint16)     # bf16 bits
    y_d = nc.dram_tensor("y", [OUT, n], f32, kind="ExternalOutput").ap()

    with tile.TileContext(nc) as tc:
        # PSUM: st 3x2 banks + ot 1 + dn/bc/yp shared 1 = 8 banks.
        with tc.tile_pool(name="const", bufs=1) as const, \
             tc.tile_pool(name="qkv", bufs=1) as qkv, \
             tc.tile_pool(name="feat", bufs=1) as feat, \
             tc.tile_pool(name="pst", bufs=3, space="PSUM") as pst, \
             tc.tile_pool(name="pot", bufs=1, space="PSUM") as pot, \
             tc.tile_pool(name="pshared", bufs=1, space="PSUM") as pshared, \
             tc.tile_pool(name="sex", bufs=8) as sex, \
             tc.tile_pool(name="sot", bufs=10) as sot, \
             tc.tile_pool(name="sbc", bufs=3) as sbc, \
             tc.tile_pool(name="smisc", bufs=4) as smisc:

            # ---- DMA: dir-a critical path first ----
            def wload(d, nm):
                t = const.tile([128, 2, E], bf16, name=nm, tag=nm)
                nc.sync.dma_start(t[:], d.rearrange("(c p) e -> p c e", p=128).bitcast(bf16))
                return t

            def vload(d, shape, nm, dt_=None, eng=None):
                t = const.tile(shape, dt_ or f32, name=nm, tag=nm)
                (eng or nc.sync).dma_start(t[:], d.bitcast(dt_) if dt_ else d)
                return t

            # dir-a critical path: wk_a (K0), wq_a (Q0) before the features
            wk = {0: wload(wk_a_d, "wka")}
            wq = {0: wload(wq_a_d, "wqa")}
            bk = {0: vload(bk_a_d, [E, 1], "bka")}
            bq = {0: vload(bq_a_d, [E, 1], "bqa")}

            fsb = {
                name: feat.tile([128, 2, n], bf16, tag=f"f{name}",
                                name=f"f_{name}")
                for name in ("a", "b")
            }
            npc = max(1, n // 512)    # 512-col pieces
            # First pieces of BOTH tensors first (K0 needs f_b[0], Q0 needs
            # f_a[0]); rest streams behind on the two HWDGE queues (sync +
            # scalar). gpsimd stays DMA-free so its tail drain is cheap.
            def fpiece(name, pc):
                d_src = f_b_d if name == "b" else f_a_d
                lo, hi = pc * (n // npc), (pc + 1) * (n // npc)
                for cc in range(2):
                    eng = nc.sync if cc == 0 else nc.scalar
                    eng.dma_start(
                        fsb[name][:, cc, lo:hi],
                        d_src[cc * 128:(cc + 1) * 128, lo:hi].bitcast(bf16),
                    )

            fpiece("b", 0)
            fpiece("a", 0)
            fpiece("b", 1)
            wv = {0: wload(wv_a_d, "wva")}
            ident = vload(ident_d, [E, E], "idt", bf16)
            ones2 = vload(ones2_d, [E, 2, 16], "on2", fp8)
            ones_r = vload(ones_r_d, [1, E], "onr", f32r)
            for pc in range(2, npc):
                fpiece("b", pc)
            for pc in range(1, npc):
                fpiece("a", pc)
            # dir-b weights + late consts
            wk[1] = wload(wk_b_d, "wkb")
            wv[1] = wload(wv_b_d, "wvb")
            wq[1] = wload(wq_b_d, "wqb")
            bk[1] = vload(bk_b_d, [E, 1], "bkb")
            bq[1] = vload(bq_b_d, [E, 1], "bqb")
            wp = const.tile([128, 2, OUT], f32r, name="wp", tag="wp")
            nc.sync.dma_start(wp[:], wp_d.rearrange("(c p) e -> p c e", p=128).bitcast(f32r))
            binv = const.tile([128, 2, 1], f32, name="binv", tag="binv")
            nc.sync.dma_start(binv[:], inv_d.rearrange("(c p) e -> p c e", p=128))
            bshf = const.tile([128, 2, 1], f32, name="bshf", tag="bshf")
            nc.sync.dma_start(bshf[:], shf_d.rearrange("(c p) e -> p c e", p=128))

            # ---- per-direction activations ----
            q_sb = {d: qkv.tile([128, n], bf16, tag=f"q{d}", name=f"q_sb{d}") for d in (0, 1)}
            k_sb = {d: qkv.tile([128, n], bf16, tag=f"k{d}", name=f"k_sb{d}") for d in (0, 1)}
            vt_sb = {d: qkv.tile([128, nkc, 128], fp8, tag=f"v{d}", name=f"vt_sb{d}")
                     for d in (0, 1)}
            vtb = {d: qkv.tile([128, nkc, 128], bf16, tag=f"vb{d}", name=f"vtb{d}")
                   for d in (0, 1)}
            vtmps = {d: feat.tile([128, n], bf16, tag=f"vtmp{d}", name=f"vtmp{d}")
                     for d in (0, 1)}

            def emit_proj(d, kind, nt):
                """One 512-wide projection tile: kind in k/v/q."""
                fq = fsb["a"] if d == 0 else fsb["b"]
                fk = fsb["b"] if d == 0 else fsb["a"]
                wt, bias, dst, src = {
                    "k": (wk[d], bk[d], k_sb[d], fk),
                    "v": (wv[d], None, vtmps[d], fk),
                    "q": (wq[d], bq[d], q_sb[d], fq),
                }[kind]
                ps = pst.tile([128, QW], f32, tag="st", name="psp")
                for cc in range(2):
                    nc.tensor.matmul(
                        ps[:],
                        wt[:, cc, :],
                        src[:, cc, nt * QW:(nt + 1) * QW],
                        start=(cc == 0),
                        stop=(cc == 1),
                    )
                with nc.allow_low_precision(reason="bf16 proj"):
                    if bias is None:
                        nc.vector.tensor_copy(
                            dst[:, nt * QW:(nt + 1) * QW], ps[:])
                    else:
                        nc.vector.tensor_scalar_add(
                            dst[:, nt * QW:(nt + 1) * QW], ps[:], bias[:])

            def emit_trans(d, g):
                """Transpose 4 v chunks -> vt fp8. dir a: PE transpose (low
                latency, feeds the very first segments). dir b: DMA xbar on
                the idle sync queue + gpsimd cast (off the PE; its segments
                start >100us later). Never touch the scalar queue: DMA
                dispatch there stalls the exp stream."""
                if d == 0:
                    ps = pst.tile([128, QW], bf16, tag="st", name="psvt")
                    for jj in range(4):
                        kc = 4 * g + jj
                        nc.tensor.transpose(
                            ps[:, jj * 128:(jj + 1) * 128],
                            vtmps[d][:, kc * 128:(kc + 1) * 128],
                            ident[:],
                        )
                    with nc.allow_low_precision(reason="fp8 VT"):
                        nc.vector.tensor_copy(
                            vt_sb[d][:, 4 * g:4 * (g + 1), :], ps[:]
                        )
                else:
                    for jj in range(4):
                        kc = 4 * g + jj
                        nc.sync.dma_start_transpose(
                            vtb[d][:, kc, :],
                            vtmps[d][:, kc * 128:(kc + 1) * 128],
                        )
                    with nc.allow_low_precision(reason="fp8 VT"):
                        nc.gpsimd.tensor_copy(
                            vt_sb[d][:, 4 * g:4 * (g + 1), :],
                            vtb[d][:, 4 * g:4 * (g + 1), :],
                        )

            # ---- attention ----
            def emit_S(d, qt, g):
                """S^T matmuls for one k-chunk group -> st psum tile."""
                qs = q_sb[d][:, qt * QW:(qt + 1) * QW]
                st = pst.tile([128, gj, QW], f32, tag="st", name="st")
                for jj in range(gj):
                    j = gj * g + jj
                    nc.tensor.matmul(
                        st[:, jj, :],
                        k_sb[d][:, j * 128:(j + 1) * 128],
                        qs,
                        start=True, stop=True,
                    )
                return st

            def emit_body(d, qt, st_pre, nxt, deferred=None):
                """exp + O/dn accumulation for one (qt, d). st_pre holds the
                2 pre-emitted S tiles (groups 0/1); the loop keeps 2 groups
                of S prefetched, crossing into segment `nxt` at the end so
                the exp engines never drain at segment boundaries.
                Returns (ot, dn, next segment's prefetched S tiles)."""
                ot = pot.tile([128, QW], f32, tag="ot", name="ot")
                dn = pshared.tile([16, QW], f32, tag="sh", name="dn")
                sts = list(st_pre)
                nxt_pre = []
                dn_pending = []   # (ex tile, group, flush-at group)
                ex_hold = []

                def emit_O(g, ext):
                    nc.tensor.matmul(
                        ot[:],
                        vt_sb[d][:, gj * g:gj * (g + 1), :],
                        ext[:],
                        start=(g == 0), stop=(g == ngrp - 1),
                        perf_mode=DR,
                    )

                def emit_dn(ext, gd):
                    nc.tensor.matmul(
                        dn[:], ones2[:], ext[:],
                        start=(gd == 0), stop=(gd == ngrp - 1),
                        perf_mode=DR,
                    )

                for g in range(ngrp):
                    if g == 2 and deferred is not None:
                        deferred()   # prev segment's bcMM/bc_copy/osb
                    if g + 2 < ngrp:
                        sts.append(emit_S(d, qt, g + 2))
                    elif nxt is not None:
                        nq, nd = nxt
                        nxt_pre.append(emit_S(nd, nq, g + 2 - ngrp))
                    # dn matmuls trail so the in-order PE never waits on exp
                    while dn_pending and dn_pending[0][2] <= g:
                        ext, gd, _ = dn_pending.pop(0)
                        emit_dn(ext, gd)
                    st_cur = sts[g]
                    ex = sex.tile([128, gj, QW], fp8, tag="ex", name="ex")
                    if (g % ngrp) in DVE_GROUPS:
                        with nc.allow_low_precision(reason="fast exp8"):
                            nc.vector.tensor_scalar(
                                ex[:].bitcast(u8), st_cur[:],
                                FE8_A, FE8_B, ALU.mult, ALU.add,
                            )
                    else:
                        nc.scalar.activation(ex[:], st_cur[:], AFT.Exp)
                    # the first O_DELAY O-matmuls are held: the fresh ot
                    # psum bank is read by the previous segment's deferred
                    # normalize mul (emitted at group 2), and this grants
                    # the slack for that handoff
                    if g < O_DELAY:
                        ex_hold.append(ex)
                    else:
                        if g == O_DELAY:
                            for gh, exh in enumerate(ex_hold):
                                emit_O(gh, exh)
                        emit_O(g, ex)
                    dn_pending.append((ex, g, g + DN_DELAY))
                for ext, gd, _ in dn_pending:
                    emit_dn(ext, gd)
                return ot, dn, nxt_pre

            def emit_tail_head(dn):
                """reciprocal of dn, right at segment end (DVE only)."""
                rc = smisc.tile([1, QW], f32, tag="rc", name="rc")
                nc.vector.reciprocal_approx_fast(rc[:], dn[0:1, :])
                rcr = smisc.tile([1, QW], f32r, tag="rcr", name="rcr")
                with nc.allow_low_precision(reason="f32r recip"):
                    nc.vector.tensor_copy(rcr[:], rc[:])
                # create the bc psum tile now so the pshared rotation order
                # stays dn(i) -> bc(i) -> dn(i+1); its matmul is deferred
                bc_ps = pshared.tile([128, QW], f32, tag="sh", name="bc")
                return rcr, bc_ps

            def emit_tail_rest(ot, rcr, bc_ps):
                """broadcast + normalize, emitted 2 groups into the next
                segment so the PE never waits on the reciprocal chain."""
                nc.tensor.matmul(
                    bc_ps[:], ones_r[:], rcr[:],
                    start=True, stop=True,
                )
                bc_sb = sbc.tile([128, QW], f32r, tag="bcs", name="bcs")
                nc.vector.tensor_copy(bc_sb[:], bc_ps[:])
                osb = sot.tile([128, QW], f32r, tag="osb", name="osb")
                with nc.allow_low_precision(reason="f32r osb"):
                    nc.vector.tensor_mul(osb[:], ot[:], bc_sb[:])
                return osb

            def emit_final(qt, ot_sbs):
                for dch in range(2):
                    yp = pshared.tile([128, QW], f32, tag="sh", name="yp")
                    for d in (0, 1):
                        nc.tensor.matmul(
                            yp[:],
                            wp[:, d, dch * 128:(dch + 1) * 128],
                            ot_sbs[d][:],
                            start=(d == 0), stop=(d == 1),
                        )
                    ysb = smisc.tile([128, QW], f32, tag="ysb", name="ysb")
                    nc.scalar.activation(
                        ysb[:], yp[:], AFT.Relu,
                        bias=bshf[:, dch, :], scale=binv[:, dch, :],
                    )
                    nc.sync.dma_start(
                        y_d[dch * 128:(dch + 1) * 128,
                            qt * QW:(qt + 1) * QW],
                        ysb[:],
                    )

            # ---- schedule ----
            # dir-a K0/Q0 + first S-tile so exp starts as soon as the first
            # feature pieces land; remaining dir-a projections follow; dir-b
            # projection tiles slip between the first dir-a segments in
            # small chunks so PE tail-stall slack absorbs them.
            emit_proj(0, "k", 0)
            emit_proj(0, "q", 0)
            st_next0 = emit_S(0, 0, 0)
            emit_proj(0, "v", 0)
            emit_trans(0, 0)
            for nt in range(1, nqt):
                emit_proj(0, "k", nt)
                emit_proj(0, "v", nt)
            emit_proj(0, "q", 1)   # needed by seg-1 prefetch inside body(0)
            for nt in range(1, nqt):
                emit_trans(0, nt)

            def fill_q0():
                # q tile nt is needed by seg-nt's prefetch, which runs inside
                # body(nt-1); filler slot i runs after body(i) -> safe for nt>=2
                for nt in range(2, nqt):
                    emit_proj(0, "q", nt)

            def fill_k1():
                for nt in range(nqt):
                    emit_proj(1, "k", nt)

            def fill_v1():
                for nt in range(nqt):
                    emit_proj(1, "v", nt)
                for nt in range(nqt):
                    emit_trans(1, nt)

            def fill_q1():
                for nt in range(nqt):
                    emit_proj(1, "q", nt)

            fillers = [fill_q0, fill_k1, fill_v1, fill_q1]
            segs = [(qt, 0) for qt in range(nqt)] + [(qt, 1) for qt in range(nqt)]

            st_pre = [st_next0, emit_S(0, 0, 1)]
            pending = {}          # qt -> {d: osb}
            deferred = None
            for i, (qt, d) in enumerate(segs):
                nxt = segs[i + 1] if i + 1 < len(segs) else None
                ot, dn, st_pre = emit_body(d, qt, st_pre, nxt,
                                           deferred=deferred)
                # finals run one segment late; they must be emitted BEFORE
                # emit_tail_head so the pshared rotation order is
                # dn(i) -> yp -> bc(i) -> dn(i+1) (bc's matmul is deferred
                # into the next segment; yp after bc would deadlock the PE)
                if d == 1 and qt > 0:
                    emit_final(qt - 1, pending.pop(qt - 1))
                rcr, bc_ps = emit_tail_head(dn)

                def deferred(ot=ot, rcr=rcr, bc_ps=bc_ps, qt=qt, d=d):
                    pending.setdefault(qt, {})[d] = emit_tail_rest(
                        ot, rcr, bc_ps)

                if i < len(fillers):
                    fillers[i]()
            deferred()
            emit_final(nqt - 1, pending.pop(nqt - 1))
    nc.compile()
    return nc


def _to_bf16_bits(x):
    u = np.ascontiguousarray(x, np.float32).view(np.uint32)
    r = ((u + 0x7FFF + ((u >> 16) & 1)) >> 16).astype(np.uint16)
    return r


def _to_fp8e4_bits(x):
    import ml_dtypes

    return np.ascontiguousarray(x, np.float32).astype(
        ml_dtypes.float8_e4m3fn).view(np.uint8)


def _host_prep(inputs, n=N):
    f_rgb = _to_bf16_bits(inputs["f_rgb"].reshape(B, C, n))
    f_pl = _to_bf16_bits(inputs["f_pl"].reshape(B, C, n))

    def T(w, scale=1.0):
        return np.ascontiguousarray(scale * np.asarray(w, np.float32).T)

    def T16(w, scale=1.0):
        return _to_bf16_bits(T(w, scale))

    wp = np.asarray(inputs["w_proj"], np.float32)
    inv = np.asarray(inputs["bn_gamma"], np.float32) / np.sqrt(
        np.asarray(inputs["bn_var"], np.float32) + 1e-5)
    shift = (np.asarray(inputs["bn_beta"], np.float32)
             - np.asarray(inputs["bn_mean"], np.float32) * inv
             + inv * (wp[:, :E] @ np.asarray(inputs["b_v_pl"], np.float32)
                      + wp[:, E:] @ np.asarray(inputs["b_v_rgb"], np.float32)))

    shared = {
        "wq_a": T16(inputs["w_q_rgb"], SCALE),
        "wk_a": T16(inputs["w_k_pl"]),
        "wv_a": T16(inputs["w_v_pl"]),
        "wq_b": T16(inputs["w_q_pl"], SCALE),
        "wk_b": T16(inputs["w_k_rgb"]),
        "wv_b": T16(inputs["w_v_rgb"]),
        "wp": T(wp),
        "bq_a": (SCALE * np.asarray(inputs["b_q_rgb"], np.float32))
        .reshape(E, 1).copy(),
        "bk_a": np.asarray(inputs["b_k_pl"], np.float32).reshape(E, 1).copy(),
        "bq_b": (SCALE * np.asarray(inputs["b_q_pl"], np.float32))
        .reshape(E, 1).copy(),
        "bk_b": np.asarray(inputs["b_k_rgb"], np.float32).reshape(E, 1).copy(),
        "bn_inv": inv.reshape(OUT, 1).copy(),
        "bn_shf": shift.reshape(OUT, 1).copy(),
        "ones2": _to_fp8e4_bits(np.ones((E, 2, 16), np.float32)),
        "ones_r": np.ones((1, E), np.float32),
        "ident": _to_bf16_bits(np.eye(E, dtype=np.float32)),
    }
    in_maps = []
    for b in range(B):
        m = dict(shared)
        m["f_a"] = f_rgb[b]
        m["f_b"] = f_pl[b]
        in_maps.append(m)
    return in_maps


def kernel(**inputs):
    from concourse import bass_utils

    if "nc" not in _CACHE:
        _CACHE["nc"] = build_nc()
    nc = _CACHE["nc"]
    in_maps = _host_prep(inputs)
    res = bass_utils.run_bass_kernel_spmd(nc, in_maps, core_ids=list(range(B)))
    out = np.stack([res.results[b]["y"] for b in range(B)], axis=0)
    return out.reshape(B, OUT, H, W).astype(np.float32)


if __name__ == "__main__":
    pass
